# revision 27
# baseline (speedup 1.0000x reference)
"""Trainium2 Bass kernel for nn_AdaptiveMultiBoxLoss (SSD multibox distillation loss).

Data-parallel over the batch dim across 8 NeuronCores.  Each core computes
partial sums (smooth-L1 loc losses, CE conf losses with hard-negative mining
via a per-row binary-search threshold top-k) over its 8 batch rows; the host
sums the 8x16 partials and performs the final division by N.

Key device-side decompositions:
  loss_c = sum_pos(lse) - sum_all conf[p, ct_p] + sum_all conf[:,0]
           - sum_pos conf[:,0] + topk(lc_masked)
  (exploits that ~98% of priors are background so the CE gather is column 0;
   the true gather sum is a one-hot trace accumulated on the TensorEngine)
  topk per row: binary-search a threshold with exact counts
  (tensor_scalar is_gt + fused accumulate), then an exact correction pass.
"""

import os
import sys

sys.path.insert(0, "/opt/trn_rl_repo")

from contextlib import ExitStack

import numpy as np

import concourse.bass as bass
import concourse.bacc as bacc
import concourse.mybir as mybir
import concourse.tile as tile
from concourse.bass_utils import run_bass_kernel_spmd

F32 = mybir.dt.float32
BF16 = mybir.dt.bfloat16
I32 = mybir.dt.int32
ALU = mybir.AluOpType
ACT = mybir.ActivationFunctionType

# ---- problem geometry (hardcoded) ----
B, P, C = 64, 8732, 81
NCORES = 8
R = B // NCORES            # 8 batch rows per core
NT = 69                    # 128-prior tiles per row (68 full + 1x28)
TFULL, TREM = 68, 28
TCOL = R * NT              # 552 columns in row-tiled layout
NFB, FBT = 3, 23           # conf stream: 3 blocks/row x 23 tiles
FBF = FBT * C              # 1863
LTT, LTFULL, LTREM = 546, 545, 96   # loc flat tiling: 546 tiles of 128 rows
LF = LTT * 4               # 2184
NPART = 16
NE_CONST = 128 * LF        # every element of the padded loc tile contributes +1
NITER = 7                  # binary search iterations (2*lc domain)
HI_INIT = 32.0

# partials columns
(COL_BT, COL_BS, COL_AT, COL_CT, COL_DT, COL_AS, COL_CS, COL_DS,
 COL_LT, COL_LS, COL_TKT, COL_TKS, COL_NP) = range(13)

STAGE = int(os.environ.get("K_STAGE", "9"))


def build_nc():
    nc = bacc.Bacc("TRN2", target_bir_lowering=False, debug=False,
                   num_devices=NCORES)

    conf_T = nc.declare_dram_parameter("conf_T", [R, P, C], F32, isOutput=False)
    conf_S = nc.declare_dram_parameter("conf_S", [R, P, C], F32, isOutput=False)
    loc_T = nc.declare_dram_parameter("loc_T", [R, P, 4], F32, isOutput=False)
    loc_S = nc.declare_dram_parameter("loc_S", [R, P, 4], F32, isOutput=False)
    loc_t = nc.declare_dram_parameter("loc_t", [R, P, 4], F32, isOutput=False)
    conf_t = nc.declare_dram_parameter("conf_t", [R, P], I32, isOutput=False)
    iota_p = nc.declare_dram_parameter("iota", [128, FBF], F32, isOutput=False)
    onesw_p = nc.declare_dram_parameter("ones8w", [8, 128], F32, isOutput=False)
    eye_p = nc.declare_dram_parameter("eye81", [81, 81], F32, isOutput=False)
    ones_p = nc.declare_dram_parameter("ones128", [128, 1], F32, isOutput=False)
    out_p = nc.declare_dram_parameter("out", [1, NPART], F32, isOutput=True)

    with tile.TileContext(nc) as tc, ExitStack() as ctx:
        cpool = ctx.enter_context(tc.tile_pool(name="consts", bufs=1))
        pers = ctx.enter_context(tc.tile_pool(name="pers", bufs=1))
        small = ctx.enter_context(tc.tile_pool(name="small", bufs=1))
        pool_cT = ctx.enter_context(tc.tile_pool(name="confT", bufs=3))
        pool_cS = ctx.enter_context(tc.tile_pool(name="confS", bufs=3))
        pool_eT = ctx.enter_context(tc.tile_pool(name="expT", bufs=2))
        pool_eS = ctx.enter_context(tc.tile_pool(name="expS", bufs=2))
        pool_eq = ctx.enter_context(tc.tile_pool(name="eq", bufs=2))
        psum = ctx.enter_context(tc.tile_pool(name="ps", bufs=4, space="PSUM"))
        pstr = ctx.enter_context(tc.tile_pool(name="tr", bufs=1, space="PSUM"))

        # ---- constants ----
        iota_sb = cpool.tile([128, FBF], F32)
        onesw_sb = cpool.tile([8, 128], F32)
        eye_sb = cpool.tile([81, 81], F32)
        ones_sb = cpool.tile([128, 1], F32)
        nc.sync.dma_start(out=iota_sb[:, :], in_=iota_p.ap())
        nc.sync.dma_start(out=onesw_sb[:, :], in_=onesw_p.ap())
        nc.sync.dma_start(out=eye_sb[:, :], in_=eye_p.ap())
        nc.sync.dma_start(out=ones_sb[:, :], in_=ones_p.ap())

        # ---- persistent tensors ----
        ctf_i = pers.tile([128, TCOL], I32)
        ctf = pers.tile([128, TCOL], F32)
        posf = pers.tile([128, TCOL], F32)
        ominus = pers.tile([128, TCOL], F32)
        vmask = pers.tile([128, TCOL], F32)
        sumexp = {x: pers.tile([128, TCOL], F32, name=f"sumexp{x}") for x in "TS"}
        conf0 = {x: pers.tile([128, TCOL], F32, name=f"conf0{x}") for x in "TS"}
        lse = {x: pers.tile([128, TCOL], F32, name=f"lse{x}") for x in "TS"}
        lcm = {x: pers.tile([128, TCOL], F32, name=f"lcm{x}") for x in "TS"}
        partials = pers.tile([128, NPART], F32)
        sgnjunk = pers.tile([128, TCOL], F32)
        sjunk = {x: pers.tile([128, TCOL], F32, name=f"sjunk{x}") for x in "TS"}

        ctfl_i = pers.tile([128, LTT], I32)
        ctfl = pers.tile([128, LTT], F32)
        posml = pers.tile([128, LTT], F32)
        locsb = {n: pers.tile([128, 2, 1096], F32, name=f"loc{n}")
                 for n in ("T", "S", "t")}
        ld = pers.tile([128, LF], F32)
        lu = pers.tile([128, LF], F32)
        lc_ = pers.tile([128, LF], F32)
        lm = pers.tile([128, LF], F32)

        nc.gpsimd.memset(partials[:, :], 0.0)

        # ---- conf_t: row-tiled layout, partition-major within each row ----
        # column r*NT + fb*FBT + j on partition p holds prior 69*p + fb*23 + j
        for r in range(R):
            nc.gpsimd.memset(ctf_i[96:128, r * NT:(r + 1) * NT], -1)
            nc.sync.dma_start(
                out=ctf_i[0:126, r * NT:(r + 1) * NT],
                in_=conf_t.ap()[r, 0:126 * NT].rearrange("(p t) -> p t", t=NT))
            nc.sync.dma_start(
                out=ctf_i[126:127, r * NT:r * NT + 38],
                in_=conf_t.ap()[r, 126 * NT:P].unsqueeze(0))
        nc.vector.tensor_copy(out=ctf[:, :], in_=ctf_i[:, :])
        nc.vector.tensor_scalar(out=posf[:, :], in0=ctf[:, :], scalar1=0.5,
                                scalar2=None, op0=ALU.is_gt)
        nc.vector.tensor_scalar(out=vmask[:, :], in0=ctf[:, :], scalar1=-0.5,
                                scalar2=2.0, op0=ALU.is_gt, op1=ALU.mult)
        nc.vector.scalar_tensor_tensor(out=ominus[:, :], in0=posf[:, :],
                                       scalar=-2.0, in1=vmask[:, :],
                                       op0=ALU.mult, op1=ALU.add)

        # num_pos per row -> k
        npp = small.tile([128, 8], F32)
        nc.vector.tensor_reduce(out=npp[:, :],
                                in_=posf[:, :].rearrange("p (r t) -> p r t", r=R),
                                axis=mybir.AxisListType.X, op=ALU.add)
        ps_np = psum.tile([8, 1], F32, tag="ps")
        nc.tensor.matmul(ps_np[:, :], lhsT=npp[:, :], rhs=ones_sb[:, :],
                         start=True, stop=True)
        np8 = small.tile([8, 1], F32)
        nc.vector.tensor_copy(out=np8[:, :], in_=ps_np[:, :])
        k8 = small.tile([8, 1], F32)
        nc.vector.tensor_scalar(out=k8[:, :], in0=np8[:, :], scalar1=3.0,
                                scalar2=float(P - 1), op0=ALU.mult, op1=ALU.min)
        nc.vector.tensor_copy(out=partials[0:8, COL_NP:COL_NP + 1], in_=np8[:, :])

        # ---- conf streaming loop ----
        pstr_t = {x: pstr.tile([81, 81], F32, name=f"pstr{x}") for x in "TS"}
        if STAGE >= 3:
            nmm = {"T": 0, "S": 0}
            fbidx = [0]
            total_mm = R * NFB * FBT
            for r in range(R):
                for fb in range(NFB):
                    cb = r * NT + fb * FBT
                    pb = fb * FBT * 128
                    ct_fb = {"T": pool_cT.tile([128, FBT, C], F32, name="ctT"),
                             "S": pool_cS.tile([128, FBT, C], F32, name="ctS")}
                    ex_fb = {"T": pool_eT.tile([128, FBT, C], BF16, name="exT"),
                             "S": pool_eS.tile([128, FBT, C], BF16, name="exS")}
                    for x, param in (("T", conf_T), ("S", conf_S)):
                        t = ct_fb[x]
                        if fbidx[0] < 3:
                            # fresh SBUF slot: clear pad partitions once so
                            # later exp() of unwritten pads stays finite
                            nc.scalar.memzero(t[96:128, :, :])
                        # src view: conf[r, 69p + fb*23 + j, c]
                        rowv = param.ap()[r, :, :]
                        main = rowv[0:126 * NT, :].rearrange(
                            "(p t) c -> p t c", t=NT)
                        nc.sync.dma_start(
                            out=t[0:126, :, :],
                            in_=main[:, fb * FBT:(fb + 1) * FBT, :])
                        if fb == 0:
                            nc.sync.dma_start(
                                out=t[126:127, :, :],
                                in_=rowv[126 * NT:126 * NT + FBT, :]
                                    .unsqueeze(0))
                        elif fb == 1:
                            nc.sync.dma_start(
                                out=t[126:127, 0:15, :],
                                in_=rowv[126 * NT + FBT:P, :].unsqueeze(0))
                        nc.scalar.activation(out=ex_fb[x][:, :, :],
                                             in_=t[:, :, 0:C], func=ACT.Exp)

                    eq_t = pool_eq.tile([128, FBT, C], F32, name="eqt")
                    ctb_view = ctf[:, cb:cb + FBT].unsqueeze(2).broadcast_to(
                        (128, FBT, C))
                    nc.vector.tensor_tensor(
                        out=eq_t[:, :, :],
                        in0=iota_sb[:, :].rearrange("p (t c) -> p t c", c=C),
                        in1=ctb_view, op=ALU.is_equal)

                    for x in "TS":
                        nc.vector.tensor_reduce(out=sumexp[x][:, cb:cb + FBT],
                                                in_=ex_fb[x][:, :, :],
                                                axis=mybir.AxisListType.X,
                                                op=ALU.add)
                        nc.vector.tensor_copy(out=conf0[x][:, cb:cb + FBT],
                                              in_=ct_fb[x][:, :, 0])
                        # one-hot trace: psum[m, c] += sum_p eq[p, m]*conf[p, c]
                        for t in range(FBT):
                            nc.tensor.matmul(pstr_t[x][:, :],
                                             lhsT=eq_t[:, t, :],
                                             rhs=ct_fb[x][:, t, 0:C],
                                             start=(nmm[x] == 0),
                                             stop=(nmm[x] == total_mm - 1))
                            nmm[x] += 1
                fbidx[0] += 1
                # per-row tail once the row's 3 blocks are done
                if fb == NFB - 1 and STAGE >= 4:
                    rc = r * NT
                    for x in "TS":
                        nc.scalar.activation(out=lse[x][:, rc:rc + NT],
                                             in_=sumexp[x][:, rc:rc + NT],
                                             func=ACT.Ln)
                        nc.vector.scalar_tensor_tensor(
                            out=sumexp[x][:, rc:rc + NT],
                            in0=conf0[x][:, rc:rc + NT], scalar=-1.0,
                            in1=lse[x][:, rc:rc + NT],
                            op0=ALU.mult, op1=ALU.add)
                        nc.vector.tensor_tensor(out=lcm[x][:, rc:rc + NT],
                                                in0=sumexp[x][:, rc:rc + NT],
                                                in1=ominus[:, rc:rc + NT],
                                                op=ALU.mult)

        # ---- per-tensor epilogue: lse, partial sums, lc_m ----
        if STAGE >= 4:
            for x, (colA, colCc, colD, colB) in (
                    ("T", (COL_AT, COL_CT, COL_DT, COL_BT)),
                    ("S", (COL_AS, COL_CS, COL_DS, COL_BS))):
                # A = sum(lse * posf)
                nc.vector.tensor_tensor(out=sgnjunk[:, :], in0=lse[x][:, :],
                                        in1=posf[:, :], op=ALU.mult)
                nc.vector.tensor_reduce(out=partials[:, colA:colA + 1],
                                        in_=sgnjunk[:, :],
                                        axis=mybir.AxisListType.X, op=ALU.add)
                # C2 = sum conf0 * vmask2  (= 2*C, pads excluded)
                nc.vector.tensor_tensor(out=sgnjunk[:, :], in0=conf0[x][:, :],
                                        in1=vmask[:, :], op=ALU.mult)
                nc.vector.tensor_reduce(out=partials[:, colCc:colCc + 1],
                                        in_=sgnjunk[:, :],
                                        axis=mybir.AxisListType.X, op=ALU.add)
                # D = sum conf0 * posf
                nc.vector.tensor_tensor(out=sgnjunk[:, :], in0=conf0[x][:, :],
                                        in1=posf[:, :], op=ALU.mult)
                nc.vector.tensor_reduce(out=partials[:, colD:colD + 1],
                                        in_=sgnjunk[:, :],
                                        axis=mybir.AxisListType.X, op=ALU.add)
                # B = trace(pstr): diag via eye mask
                nc.vector.tensor_tensor(out=sgnjunk[0:81, 0:81],
                                        in0=pstr_t[x][:, :], in1=eye_sb[:, :],
                                        op=ALU.mult)
                nc.vector.tensor_reduce(out=partials[0:81, colB:colB + 1],
                                        in_=sgnjunk[0:81, 0:81],
                                        axis=mybir.AxisListType.X, op=ALU.add)

        # ---- binary search for per-row top-k count thresholds ----
        # natural layout: per-row thresholds broadcast to [128, 8] via
        # diag(t) matmul, counts via 8 per-row tensor_scalar+accum ops.
        lo = {x: small.tile([8, 1], F32, name=f"lo{x}") for x in "TS"}
        hi = {x: small.tile([8, 1], F32, name=f"hi{x}") for x in "TS"}
        tmid = {x: small.tile([8, 1], F32, name=f"tm{x}") for x in "TS"}
        ge = {x: small.tile([8, 1], I32, name=f"ge{x}") for x in "TS"}
        gei = {x: small.tile([8, 1], I32, name=f"gei{x}") for x in "TS"}
        s8 = {x: small.tile([8, 1], F32, name=f"s8{x}") for x in "TS"}
        diag8 = {x: small.tile([8, 8], F32, name=f"dg{x}") for x in "TS"}
        trep = {x: small.tile([128, 8], F32, name=f"trep{x}") for x in "TS"}
        cnt8 = {x: small.tile([128, 8], F32, name=f"cnt8{x}") for x in "TS"}
        s8p = {x: small.tile([128, 8], F32, name=f"s8p{x}") for x in "TS"}
        ns8 = {x: small.tile([8, 2], F32, name=f"ns8{x}") for x in "TS"}
        tk = {x: small.tile([8, 1], F32, name=f"tk{x}") for x in "TS"}

        def bcast_rows(vec8, x):
            # trep[q, r] = vec8[r]  for all partitions q
            nc.vector.tensor_tensor(out=diag8[x][:, :], in0=eye_sb[0:8, 0:8],
                                    in1=vec8[:, :].broadcast_to((8, 8)),
                                    op=ALU.mult)
            psA = psum.tile([128, 8], F32, name="psA", tag="ps")
            nc.tensor.matmul(psA[:, :], lhsT=onesw_sb[:, :],
                             rhs=diag8[x][:, :], start=True, stop=True)
            nc.vector.tensor_copy(out=trep[x][:, :], in_=psA[:, :])

        def row_counts(x, src_tile, out128x8):
            for r in range(R):
                nc.vector.tensor_scalar(
                    out=sjunk[x][:, r * NT:(r + 1) * NT],
                    in0=src_tile[:, r * NT:(r + 1) * NT],
                    scalar1=trep[x][:, r:r + 1], scalar2=None,
                    op0=ALU.is_gt, op1=ALU.add,
                    accum_out=out128x8[:, r:r + 1])

        if STAGE >= 5:
            for x in "TS":
                nc.gpsimd.memset(lo[x][:, :], 0.0)
                nc.gpsimd.memset(hi[x][:, :], HI_INIT)
            for it in range(NITER):
                for x in "TS":
                    nc.vector.tensor_tensor(out=tmid[x][:, :], in0=lo[x][:, :],
                                            in1=hi[x][:, :], op=ALU.add)
                    nc.vector.tensor_scalar(out=tmid[x][:, :], in0=tmid[x][:, :],
                                            scalar1=0.5, scalar2=None,
                                            op0=ALU.mult)
                    bcast_rows(tmid[x], x)
                    row_counts(x, lcm[x], cnt8[x])
                    psB = psum.tile([8, 1], F32, name="psB", tag="ps")
                    nc.tensor.matmul(psB[:, :], lhsT=cnt8[x][:, :],
                                     rhs=ones_sb[:, :], start=True, stop=True)
                    nc.vector.tensor_copy(out=s8[x][:, :], in_=psB[:, :])
                    nc.vector.tensor_tensor(out=ge[x][:, :], in0=s8[x][:, :],
                                            in1=k8[:, :], op=ALU.is_ge)
                    nc.vector.copy_predicated(out=lo[x][:, :], mask=ge[x][:, :],
                                              data=tmid[x][:, :])
                    nc.vector.tensor_scalar(out=gei[x][:, :], in0=ge[x][:, :],
                                            scalar1=1, scalar2=None,
                                            op0=ALU.bitwise_xor)
                    nc.vector.copy_predicated(out=hi[x][:, :], mask=gei[x][:, :],
                                              data=tmid[x][:, :])

        # ---- exact pass at t* = lo ----
        if STAGE >= 6:
            for x, colk in (("T", COL_TKT), ("S", COL_TKS)):
                bcast_rows(lo[x], x)
                row_counts(x, lcm[x], cnt8[x])
                nc.vector.tensor_tensor(out=lse[x][:, :], in0=lcm[x][:, :],
                                        in1=sjunk[x][:, :], op=ALU.mult)
                nc.vector.tensor_reduce(
                    out=s8p[x][:, :],
                    in_=lse[x][:, :].rearrange("p (r t) -> p r t", r=R),
                    axis=mybir.AxisListType.X, op=ALU.add)
                psC = psum.tile([8, 2], F32, name="psC", tag="ps")
                nc.tensor.matmul(psC[:, 0:1], lhsT=cnt8[x][:, :],
                                 rhs=ones_sb[:, :], start=True, stop=True)
                nc.tensor.matmul(psC[:, 1:2], lhsT=s8p[x][:, :],
                                 rhs=ones_sb[:, :], start=True, stop=True)
                nc.vector.tensor_copy(out=ns8[x][:, :], in_=psC[:, :])
                # topk = S* + (k - n*) * t*
                nc.vector.tensor_tensor(out=tk[x][:, :], in0=k8[:, :],
                                        in1=ns8[x][:, 0:1], op=ALU.subtract)
                nc.vector.tensor_tensor(out=tk[x][:, :], in0=tk[x][:, :],
                                        in1=lo[x][:, :], op=ALU.mult)
                nc.vector.tensor_tensor(out=tk[x][:, :], in0=tk[x][:, :],
                                        in1=ns8[x][:, 1:2], op=ALU.add)
                nc.vector.tensor_scalar(out=tk[x][:, :], in0=tk[x][:, :],
                                        scalar1=0.5, scalar2=None, op0=ALU.mult)
                nc.vector.tensor_copy(out=partials[0:8, colk:colk + 1],
                                      in_=tk[x][:, :])

        # ---- conf_t flat layout (for loc masking) ----
        # gates: loc DMAs wait for end-of-streaming; the loc vector chain
        # waits for the search to finish so it fills the tail instead of
        # stalling the vector FIFO mid-kernel.
        if STAGE >= 6:
            for n in ("T", "S", "t"):
                nc.vector.tensor_copy(out=locsb[n][0:8, 0, 0:1],
                                      in_=lcm["S"][0:8, TCOL - 1:TCOL])
            nc.vector.tensor_copy(out=ld[0:8, 0:1], in_=tk["S"][:, :])
        ct_flat = conf_t.ap().rearrange("r p -> (r p)")
        nc.gpsimd.memset(ctfl_i[96:128, :], -1)
        nc.sync.dma_start(
            out=ctfl_i[0:127, :],
            in_=ct_flat[0:127 * LTT].rearrange("(p t) -> p t", t=LTT))
        nc.sync.dma_start(
            out=ctfl_i[127:128, 0:R * P - 127 * LTT],
            in_=ct_flat[127 * LTT:R * P].unsqueeze(0))
        nc.vector.tensor_copy(out=ctfl[:, :], in_=ctfl_i[:, :])
        nc.vector.tensor_scalar(out=posml[:, :], in0=ctfl[:, :], scalar1=0.5,
                                scalar2=None, op0=ALU.is_gt)

        # ---- loc DMAs ----
        for name, param in (("T", loc_T), ("S", loc_S), ("t", loc_t)):
            dst = locsb[name]
            flat = param.ap().rearrange("r p f -> (r p) f")
            nc.gpsimd.memset(dst[96:128, :, :], 0.0)
            for a in range(2):
                nc.sync.dma_start(
                    out=dst[0:127, a, 0:1092],
                    in_=flat[0:127 * LTT, :]
                        .rearrange("(p a j) f -> p a (j f)", a=2, j=273)[:, a, :])
            nc.sync.dma_start(
                out=dst[127:128, 0:1, 0:1092],
                in_=flat[127 * LTT:127 * LTT + 273, :]
                    .rearrange("(a j) f -> a (j f)", a=1).unsqueeze(0))
            nc.sync.dma_start(
                out=dst[127:128, 1:2, 0:964],
                in_=flat[127 * LTT + 273:R * P, :]
                    .rearrange("(a j) f -> a (j f)", a=1).unsqueeze(0))

        # ---- loc smooth-L1 (masked, sum) ----
        # per element: 0.5*min(u,1)^2 + max(u,1) - 1 with u = |loc - loc_t|*pos
        # masked/pad elements contribute exactly +1, subtracted as NE_CONST.
        if STAGE >= 2:
            posml4 = (posml[:, :].rearrange("p (a j) -> p a j", a=2)
                      .unsqueeze(3).broadcast_to((128, 2, 273, 4)))
            for x, col in (("T", COL_LT), ("S", COL_LS)):
                nc.vector.tensor_tensor(
                    out=ld[:, :].rearrange("p (a e) -> p a e", a=2),
                    in0=locsb[x][:, :, 0:1092],
                    in1=locsb["t"][:, :, 0:1092],
                    op=ALU.subtract)
                nc.vector.tensor_tensor(
                    out=lu[:, :].rearrange("p (a j f) -> p a j f", a=2, j=273),
                    in0=ld[:, :].rearrange("p (a j f) -> p a j f", a=2, j=273),
                    in1=posml4, op=ALU.mult)
                nc.scalar.activation(out=lu[:, :], in_=lu[:, :], func=ACT.Abs)
                nc.vector.tensor_scalar(out=lc_[:, :], in0=lu[:, :], scalar1=1.0,
                                        scalar2=None, op0=ALU.min)
                nc.vector.tensor_scalar(out=lm[:, :], in0=lu[:, :], scalar1=1.0,
                                        scalar2=None, op0=ALU.max)
                nc.scalar.activation(out=lc_[:, :], in_=lc_[:, :], func=ACT.Square,
                                     scale=float(1.0 / np.sqrt(2.0)))
                nc.vector.tensor_tensor(out=ld[:, :], in0=lc_[:, :],
                                        in1=lm[:, :], op=ALU.add)
                nc.vector.tensor_reduce(out=partials[:, col:col + 1],
                                        in_=ld[:, :],
                                        axis=mybir.AxisListType.X, op=ALU.add)


        # ---- final partition reduce of partials -> out ----
        psF = psum.tile([1, NPART], F32, name="psF", tag="ps")
        nc.tensor.matmul(psF[:, :], lhsT=ones_sb[:, :], rhs=partials[:, :],
                         start=True, stop=True)
        fin = small.tile([1, NPART], F32)
        nc.vector.tensor_copy(out=fin[:, :], in_=psF[:, :])
        nc.sync.dma_start(out=out_p.ap(), in_=fin[:, :])
    nc.finalize()
    return nc


_NC_CACHE = None


def _get_nc():
    global _NC_CACHE
    if _NC_CACHE is None:
        _NC_CACHE = build_nc()
    return _NC_CACHE


def _host_consts():
    iota = np.ascontiguousarray(
        np.tile(np.arange(C, dtype=np.float32), FBT)[None, :].repeat(128, 0))
    ones8w = np.ones((8, 128), np.float32)
    eye81 = np.eye(81, dtype=np.float32)
    ones = np.ones((128, 1), np.float32)
    return iota, ones8w, eye81, ones


def _build_in_maps(inputs):
    conf_T = np.ascontiguousarray(np.asarray(inputs["conf_dataT"], np.float32))
    conf_S = np.ascontiguousarray(np.asarray(inputs["conf_dataS"], np.float32))
    loc_T = np.ascontiguousarray(np.asarray(inputs["loc_dataT"], np.float32))
    loc_S = np.ascontiguousarray(np.asarray(inputs["loc_dataS"], np.float32))
    loc_t = np.ascontiguousarray(np.asarray(inputs["loc_t"], np.float32))
    ct = np.ascontiguousarray(np.asarray(inputs["conf_t"], np.int32))
    iota, ones8w, eye81, ones = _host_consts()
    in_maps = []
    for d in range(NCORES):
        sl = slice(d * R, (d + 1) * R)
        in_maps.append({
            "conf_T": conf_T[sl], "conf_S": conf_S[sl],
            "loc_T": loc_T[sl], "loc_S": loc_S[sl], "loc_t": loc_t[sl],
            "conf_t": ct[sl],
            "iota": iota, "ones8w": ones8w,
            "eye81": eye81, "ones128": ones,
        })
    return in_maps


def _combine(parts):
    S = parts.astype(np.float64).sum(axis=0)
    loss_cT = S[COL_AT] - S[COL_BT] + S[COL_CT] / 2 - S[COL_DT] + S[COL_TKT]
    loss_cS = S[COL_AS] - S[COL_BS] + S[COL_CS] / 2 - S[COL_DS] + S[COL_TKS]
    loss_lT = S[COL_LT] - NCORES * NE_CONST
    loss_lS = S[COL_LS] - NCORES * NE_CONST
    N = S[COL_NP]
    return np.array([loss_lT / N, loss_cT / N, loss_lS / N, loss_cS / N],
                    np.float32)


def run_on_hw(inputs, trace=False, **kw):
    nc = _get_nc()
    in_maps = _build_in_maps(inputs)
    res = run_bass_kernel_spmd(nc, in_maps, core_ids=list(range(NCORES)),
                               trace=trace, **kw)
    parts = np.stack([np.asarray(r["out"]).reshape(NPART) for r in res.results])
    return _combine(parts), res


def kernel(**inputs) -> np.ndarray:
    out, _ = run_on_hw(inputs, trace=False)
    return out


# revision 28
# speedup vs baseline: 1.1684x; 1.1684x over previous
"""Trainium2 Bass kernel for nn_AdaptiveMultiBoxLoss (SSD multibox distillation loss).

Data-parallel over the batch dim across 8 NeuronCores.  Each core computes
partial sums (smooth-L1 loc losses, CE conf losses with hard-negative mining
via a per-row binary-search threshold top-k) over its 8 batch rows; the host
sums the 8x16 partials and performs the final division by N.

Key device-side decompositions:
  loss_c = sum_pos(lse) - sum_all conf[p, ct_p] + sum_all conf[:,0]
           - sum_pos conf[:,0] + topk(lc_masked)
  (exploits that ~98% of priors are background so the CE gather is column 0;
   the true gather sum is a one-hot trace accumulated on the TensorEngine)
  topk per row: binary-search a threshold with exact counts
  (tensor_scalar is_gt + fused accumulate), then an exact correction pass.
"""

import os
import sys

sys.path.insert(0, "/opt/trn_rl_repo")

from contextlib import ExitStack

import numpy as np

import concourse.bass as bass
import concourse.bacc as bacc
import concourse.mybir as mybir
import concourse.tile as tile
from concourse.bass_utils import run_bass_kernel_spmd

F32 = mybir.dt.float32
BF16 = mybir.dt.bfloat16
I32 = mybir.dt.int32
ALU = mybir.AluOpType
ACT = mybir.ActivationFunctionType

# ---- problem geometry (hardcoded) ----
B, P, C = 64, 8732, 81
NCORES = 8
R = B // NCORES            # 8 batch rows per core
NT = 69                    # 128-prior tiles per row (68 full + 1x28)
TFULL, TREM = 68, 28
TCOL = R * NT              # 552 columns in row-tiled layout
NFB, FBT = 3, 23           # conf stream: 3 blocks/row x 23 tiles
FBF = FBT * C              # 1863
LTT, LTFULL, LTREM = 546, 545, 96   # loc flat tiling: 546 tiles of 128 rows
LF = LTT * 4               # 2184
NPART = 16
NE_CONST = 128 * LF        # every element of the padded loc tile contributes +1
NITER = 7                  # binary search iterations (2*lc domain)
HI_INIT = 32.0

# partials columns
(COL_BT, COL_BS, COL_AT, COL_CT, COL_DT, COL_AS, COL_CS, COL_DS,
 COL_LT, COL_LS, COL_TKT, COL_TKS, COL_NP) = range(13)

STAGE = int(os.environ.get("K_STAGE", "9"))


def build_nc():
    nc = bacc.Bacc("TRN2", target_bir_lowering=False, debug=False,
                   num_devices=NCORES)

    conf_T = nc.declare_dram_parameter("conf_T", [R, P, C], F32, isOutput=False)
    conf_S = nc.declare_dram_parameter("conf_S", [R, P, C], F32, isOutput=False)
    loc_T = nc.declare_dram_parameter("loc_T", [128 * LTT, 4], F32, isOutput=False)
    loc_S = nc.declare_dram_parameter("loc_S", [128 * LTT, 4], F32, isOutput=False)
    loc_t = nc.declare_dram_parameter("loc_t", [128 * LTT, 4], F32, isOutput=False)
    ctp = nc.declare_dram_parameter("ctp", [128 * LTT], I32, isOutput=False)
    conf_t = nc.declare_dram_parameter("conf_t", [R, P], I32, isOutput=False)
    iota_p = nc.declare_dram_parameter("iota", [128, FBF], F32, isOutput=False)
    onesw_p = nc.declare_dram_parameter("ones8w", [8, 128], F32, isOutput=False)
    eye_p = nc.declare_dram_parameter("eye81", [81, 81], F32, isOutput=False)
    ones_p = nc.declare_dram_parameter("ones128", [128, 1], F32, isOutput=False)
    out_p = nc.declare_dram_parameter("out", [1, NPART], F32, isOutput=True)

    with tile.TileContext(nc) as tc, ExitStack() as ctx:
        cpool = ctx.enter_context(tc.tile_pool(name="consts", bufs=1))
        pers = ctx.enter_context(tc.tile_pool(name="pers", bufs=1))
        small = ctx.enter_context(tc.tile_pool(name="small", bufs=1))
        pool_cT = ctx.enter_context(tc.tile_pool(name="confT", bufs=3))
        pool_cS = ctx.enter_context(tc.tile_pool(name="confS", bufs=3))
        pool_eT = ctx.enter_context(tc.tile_pool(name="expT", bufs=2))
        pool_eS = ctx.enter_context(tc.tile_pool(name="expS", bufs=2))
        pool_eq = ctx.enter_context(tc.tile_pool(name="eq", bufs=2))
        psum = ctx.enter_context(tc.tile_pool(name="ps", bufs=4, space="PSUM"))
        pstr = ctx.enter_context(tc.tile_pool(name="tr", bufs=1, space="PSUM"))

        # ---- constants ----
        iota_sb = cpool.tile([128, FBF], F32)
        onesw_sb = cpool.tile([8, 128], F32)
        eye_sb = cpool.tile([81, 81], F32)
        ones_sb = cpool.tile([128, 1], F32)
        nc.sync.dma_start(out=iota_sb[:, :], in_=iota_p.ap())
        nc.sync.dma_start(out=onesw_sb[:, :], in_=onesw_p.ap())
        nc.sync.dma_start(out=eye_sb[:, :], in_=eye_p.ap())
        nc.sync.dma_start(out=ones_sb[:, :], in_=ones_p.ap())

        # ---- persistent tensors ----
        ctf_i = pers.tile([128, TCOL], I32)
        ctf = pers.tile([128, TCOL], F32)
        posf = pers.tile([128, TCOL], F32)
        ominus = pers.tile([128, TCOL], F32)
        vmask = pers.tile([128, TCOL], F32)
        sumexp = {x: pers.tile([128, TCOL], F32, name=f"sumexp{x}") for x in "TS"}
        conf0 = {x: pers.tile([128, TCOL], F32, name=f"conf0{x}") for x in "TS"}
        lse = {x: pers.tile([128, TCOL], F32, name=f"lse{x}") for x in "TS"}
        lcm = {x: pers.tile([128, TCOL], F32, name=f"lcm{x}") for x in "TS"}
        partials = pers.tile([128, NPART], F32)
        sgnjunk = pers.tile([128, TCOL], F32)
        sjunk = {x: pers.tile([128, TCOL], F32, name=f"sjunk{x}") for x in "TS"}

        ctfl_i = pers.tile([128, LTT], I32)
        ctfl = pers.tile([128, LTT], F32)
        posml = pers.tile([128, LTT], F32)
        locsb = {n: pers.tile([128, 2, 1096], F32, name=f"loc{n}")
                 for n in ("T", "S", "t")}
        ld = pers.tile([128, LF], F32)
        lu = pers.tile([128, LF], F32)
        lc_ = pers.tile([128, LF], F32)
        lm = pers.tile([128, LF], F32)

        nc.gpsimd.memset(partials[:, :], 0.0)

        # ---- conf_t: row-tiled layout, partition-major within each row ----
        # column r*NT + fb*FBT + j on partition p holds prior 69*p + fb*23 + j
        for r in range(R):
            nc.gpsimd.memset(ctf_i[96:128, r * NT:(r + 1) * NT], -1)
            nc.sync.dma_start(
                out=ctf_i[0:126, r * NT:(r + 1) * NT],
                in_=conf_t.ap()[r, 0:126 * NT].rearrange("(p t) -> p t", t=NT))
            nc.sync.dma_start(
                out=ctf_i[126:127, r * NT:r * NT + 38],
                in_=conf_t.ap()[r, 126 * NT:P].unsqueeze(0))
        nc.vector.tensor_copy(out=ctf[:, :], in_=ctf_i[:, :])
        nc.vector.tensor_scalar(out=posf[:, :], in0=ctf[:, :], scalar1=0.5,
                                scalar2=None, op0=ALU.is_gt)
        nc.vector.tensor_scalar(out=vmask[:, :], in0=ctf[:, :], scalar1=-0.5,
                                scalar2=2.0, op0=ALU.is_gt, op1=ALU.mult)
        nc.vector.scalar_tensor_tensor(out=ominus[:, :], in0=posf[:, :],
                                       scalar=-2.0, in1=vmask[:, :],
                                       op0=ALU.mult, op1=ALU.add)

        # num_pos per row -> k
        npp = small.tile([128, 8], F32)
        nc.vector.tensor_reduce(out=npp[:, :],
                                in_=posf[:, :].rearrange("p (r t) -> p r t", r=R),
                                axis=mybir.AxisListType.X, op=ALU.add)
        ps_np = psum.tile([8, 1], F32, tag="ps")
        nc.tensor.matmul(ps_np[:, :], lhsT=npp[:, :], rhs=ones_sb[:, :],
                         start=True, stop=True)
        np8 = small.tile([8, 1], F32)
        nc.vector.tensor_copy(out=np8[:, :], in_=ps_np[:, :])
        k8 = small.tile([8, 1], F32)
        nc.vector.tensor_scalar(out=k8[:, :], in0=np8[:, :], scalar1=3.0,
                                scalar2=float(P - 1), op0=ALU.mult, op1=ALU.min)
        nc.vector.tensor_copy(out=partials[0:8, COL_NP:COL_NP + 1], in_=np8[:, :])

        # ---- conf streaming loop ----
        pstr_t = {x: pstr.tile([81, 81], F32, name=f"pstr{x}") for x in "TS"}
        if STAGE >= 3:
            nmm = {"T": 0, "S": 0}
            fbidx = [0]
            total_mm = R * NFB * FBT
            for r in range(R):
                for fb in range(NFB):
                    cb = r * NT + fb * FBT
                    pb = fb * FBT * 128
                    ct_fb = {"T": pool_cT.tile([128, FBT, C], F32, name="ctT"),
                             "S": pool_cS.tile([128, FBT, C], F32, name="ctS")}
                    ex_fb = {"T": pool_eT.tile([128, FBT, C], BF16, name="exT"),
                             "S": pool_eS.tile([128, FBT, C], BF16, name="exS")}
                    for x, param in (("T", conf_T), ("S", conf_S)):
                        t = ct_fb[x]
                        if fbidx[0] < 3:
                            # fresh SBUF slot: clear pad partitions once so
                            # later exp() of unwritten pads stays finite
                            nc.scalar.memzero(t[96:128, :, :])
                        # src view: conf[r, 69p + fb*23 + j, c]
                        rowv = param.ap()[r, :, :]
                        main = rowv[0:126 * NT, :].rearrange(
                            "(p t) c -> p t c", t=NT)
                        nc.sync.dma_start(
                            out=t[0:126, :, :],
                            in_=main[:, fb * FBT:(fb + 1) * FBT, :])
                        if fb == 0:
                            nc.sync.dma_start(
                                out=t[126:127, :, :],
                                in_=rowv[126 * NT:126 * NT + FBT, :]
                                    .unsqueeze(0))
                        elif fb == 1:
                            nc.sync.dma_start(
                                out=t[126:127, 0:15, :],
                                in_=rowv[126 * NT + FBT:P, :].unsqueeze(0))
                        nc.scalar.activation(out=ex_fb[x][:, :, :],
                                             in_=t[:, :, 0:C], func=ACT.Exp)

                    eq_t = pool_eq.tile([128, FBT, C], F32, name="eqt")
                    ctb_view = ctf[:, cb:cb + FBT].unsqueeze(2).broadcast_to(
                        (128, FBT, C))
                    nc.vector.tensor_tensor(
                        out=eq_t[:, :, :],
                        in0=iota_sb[:, :].rearrange("p (t c) -> p t c", c=C),
                        in1=ctb_view, op=ALU.is_equal)

                    for x in "TS":
                        nc.vector.tensor_reduce(out=sumexp[x][:, cb:cb + FBT],
                                                in_=ex_fb[x][:, :, :],
                                                axis=mybir.AxisListType.X,
                                                op=ALU.add)
                        nc.vector.tensor_copy(out=conf0[x][:, cb:cb + FBT],
                                              in_=ct_fb[x][:, :, 0])
                        # one-hot trace: psum[m, c] += sum_p eq[p, m]*conf[p, c]
                        for t in range(FBT):
                            nc.tensor.matmul(pstr_t[x][:, :],
                                             lhsT=eq_t[:, t, :],
                                             rhs=ct_fb[x][:, t, 0:C],
                                             start=(nmm[x] == 0),
                                             stop=(nmm[x] == total_mm - 1))
                            nmm[x] += 1
                fbidx[0] += 1
                # per-row tail once the row's 3 blocks are done
                if fb == NFB - 1 and STAGE >= 4:
                    rc = r * NT
                    for x in "TS":
                        nc.scalar.activation(out=lse[x][:, rc:rc + NT],
                                             in_=sumexp[x][:, rc:rc + NT],
                                             func=ACT.Ln)
                        nc.vector.scalar_tensor_tensor(
                            out=sumexp[x][:, rc:rc + NT],
                            in0=conf0[x][:, rc:rc + NT], scalar=-1.0,
                            in1=lse[x][:, rc:rc + NT],
                            op0=ALU.mult, op1=ALU.add)
                        nc.vector.tensor_tensor(out=lcm[x][:, rc:rc + NT],
                                                in0=sumexp[x][:, rc:rc + NT],
                                                in1=ominus[:, rc:rc + NT],
                                                op=ALU.mult)

        # ---- per-tensor epilogue: lse, partial sums, lc_m ----
        if STAGE >= 4:
            for x, (colA, colCc, colD, colB) in (
                    ("T", (COL_AT, COL_CT, COL_DT, COL_BT)),
                    ("S", (COL_AS, COL_CS, COL_DS, COL_BS))):
                # A = sum(lse * posf)
                nc.vector.tensor_tensor(out=sgnjunk[:, :], in0=lse[x][:, :],
                                        in1=posf[:, :], op=ALU.mult)
                nc.vector.tensor_reduce(out=partials[:, colA:colA + 1],
                                        in_=sgnjunk[:, :],
                                        axis=mybir.AxisListType.X, op=ALU.add)
                # C2 = sum conf0 * vmask2  (= 2*C, pads excluded)
                nc.vector.tensor_tensor(out=sgnjunk[:, :], in0=conf0[x][:, :],
                                        in1=vmask[:, :], op=ALU.mult)
                nc.vector.tensor_reduce(out=partials[:, colCc:colCc + 1],
                                        in_=sgnjunk[:, :],
                                        axis=mybir.AxisListType.X, op=ALU.add)
                # D = sum conf0 * posf
                nc.vector.tensor_tensor(out=sgnjunk[:, :], in0=conf0[x][:, :],
                                        in1=posf[:, :], op=ALU.mult)
                nc.vector.tensor_reduce(out=partials[:, colD:colD + 1],
                                        in_=sgnjunk[:, :],
                                        axis=mybir.AxisListType.X, op=ALU.add)
                # B = trace(pstr): diag via eye mask
                nc.vector.tensor_tensor(out=sgnjunk[0:81, 0:81],
                                        in0=pstr_t[x][:, :], in1=eye_sb[:, :],
                                        op=ALU.mult)
                nc.vector.tensor_reduce(out=partials[0:81, colB:colB + 1],
                                        in_=sgnjunk[0:81, 0:81],
                                        axis=mybir.AxisListType.X, op=ALU.add)

        # ---- binary search for per-row top-k count thresholds ----
        # natural layout: per-row thresholds broadcast to [128, 8] via
        # diag(t) matmul, counts via 8 per-row tensor_scalar+accum ops.
        lo = {x: small.tile([8, 1], F32, name=f"lo{x}") for x in "TS"}
        hi = {x: small.tile([8, 1], F32, name=f"hi{x}") for x in "TS"}
        tmid = {x: small.tile([8, 1], F32, name=f"tm{x}") for x in "TS"}
        ge = {x: small.tile([8, 1], I32, name=f"ge{x}") for x in "TS"}
        gei = {x: small.tile([8, 1], I32, name=f"gei{x}") for x in "TS"}
        s8 = {x: small.tile([8, 1], F32, name=f"s8{x}") for x in "TS"}
        diag8 = {x: small.tile([8, 8], F32, name=f"dg{x}") for x in "TS"}
        trep = {x: small.tile([128, 8], F32, name=f"trep{x}") for x in "TS"}
        cnt8 = {x: small.tile([128, 8], F32, name=f"cnt8{x}") for x in "TS"}
        s8p = {x: small.tile([128, 8], F32, name=f"s8p{x}") for x in "TS"}
        ns8 = {x: small.tile([8, 2], F32, name=f"ns8{x}") for x in "TS"}
        tk = {x: small.tile([8, 1], F32, name=f"tk{x}") for x in "TS"}

        def bcast_rows(vec8, x):
            # trep[q, r] = vec8[r]  for all partitions q
            nc.vector.tensor_tensor(out=diag8[x][:, :], in0=eye_sb[0:8, 0:8],
                                    in1=vec8[:, :].broadcast_to((8, 8)),
                                    op=ALU.mult)
            psA = psum.tile([128, 8], F32, name="psA", tag="ps")
            nc.tensor.matmul(psA[:, :], lhsT=onesw_sb[:, :],
                             rhs=diag8[x][:, :], start=True, stop=True)
            nc.vector.tensor_copy(out=trep[x][:, :], in_=psA[:, :])

        def row_counts(x, src_tile, out128x8):
            for r in range(R):
                nc.vector.tensor_scalar(
                    out=sjunk[x][:, r * NT:(r + 1) * NT],
                    in0=src_tile[:, r * NT:(r + 1) * NT],
                    scalar1=trep[x][:, r:r + 1], scalar2=None,
                    op0=ALU.is_gt, op1=ALU.add,
                    accum_out=out128x8[:, r:r + 1])

        if STAGE >= 5:
            for x in "TS":
                nc.gpsimd.memset(lo[x][:, :], 0.0)
                nc.gpsimd.memset(hi[x][:, :], HI_INIT)
            for it in range(NITER):
                for x in "TS":
                    nc.vector.tensor_tensor(out=tmid[x][:, :], in0=lo[x][:, :],
                                            in1=hi[x][:, :], op=ALU.add)
                    nc.vector.tensor_scalar(out=tmid[x][:, :], in0=tmid[x][:, :],
                                            scalar1=0.5, scalar2=None,
                                            op0=ALU.mult)
                    bcast_rows(tmid[x], x)
                    row_counts(x, lcm[x], cnt8[x])
                    psB = psum.tile([8, 1], F32, name="psB", tag="ps")
                    nc.tensor.matmul(psB[:, :], lhsT=cnt8[x][:, :],
                                     rhs=ones_sb[:, :], start=True, stop=True)
                    nc.vector.tensor_copy(out=s8[x][:, :], in_=psB[:, :])
                    nc.vector.tensor_tensor(out=ge[x][:, :], in0=s8[x][:, :],
                                            in1=k8[:, :], op=ALU.is_ge)
                    nc.vector.copy_predicated(out=lo[x][:, :], mask=ge[x][:, :],
                                              data=tmid[x][:, :])
                    nc.vector.tensor_scalar(out=gei[x][:, :], in0=ge[x][:, :],
                                            scalar1=1, scalar2=None,
                                            op0=ALU.bitwise_xor)
                    nc.vector.copy_predicated(out=hi[x][:, :], mask=gei[x][:, :],
                                              data=tmid[x][:, :])

        # ---- exact pass at t* = lo ----
        if STAGE >= 6:
            for x, colk in (("T", COL_TKT), ("S", COL_TKS)):
                bcast_rows(lo[x], x)
                row_counts(x, lcm[x], cnt8[x])
                nc.vector.tensor_tensor(out=lse[x][:, :], in0=lcm[x][:, :],
                                        in1=sjunk[x][:, :], op=ALU.mult)
                nc.vector.tensor_reduce(
                    out=s8p[x][:, :],
                    in_=lse[x][:, :].rearrange("p (r t) -> p r t", r=R),
                    axis=mybir.AxisListType.X, op=ALU.add)
                psC = psum.tile([8, 2], F32, name="psC", tag="ps")
                nc.tensor.matmul(psC[:, 0:1], lhsT=cnt8[x][:, :],
                                 rhs=ones_sb[:, :], start=True, stop=True)
                nc.tensor.matmul(psC[:, 1:2], lhsT=s8p[x][:, :],
                                 rhs=ones_sb[:, :], start=True, stop=True)
                nc.vector.tensor_copy(out=ns8[x][:, :], in_=psC[:, :])
                # topk = S* + (k - n*) * t*
                nc.vector.tensor_tensor(out=tk[x][:, :], in0=k8[:, :],
                                        in1=ns8[x][:, 0:1], op=ALU.subtract)
                nc.vector.tensor_tensor(out=tk[x][:, :], in0=tk[x][:, :],
                                        in1=lo[x][:, :], op=ALU.mult)
                nc.vector.tensor_tensor(out=tk[x][:, :], in0=tk[x][:, :],
                                        in1=ns8[x][:, 1:2], op=ALU.add)
                nc.vector.tensor_scalar(out=tk[x][:, :], in0=tk[x][:, :],
                                        scalar1=0.5, scalar2=None, op0=ALU.mult)
                nc.vector.tensor_copy(out=partials[0:8, colk:colk + 1],
                                      in_=tk[x][:, :])

        # ---- conf_t flat layout (for loc masking) ----
        # gates: loc DMAs wait for end-of-streaming; the loc vector chain
        # waits for the search to finish so it fills the tail instead of
        # stalling the vector FIFO mid-kernel.
        if STAGE >= 6:
            for n in ("T", "S", "t"):
                nc.vector.tensor_copy(out=locsb[n][0:8, 0, 0:1],
                                      in_=lcm["S"][0:8, TCOL - 1:TCOL])
            nc.vector.tensor_copy(out=ld[0:8, 0:1], in_=tk["S"][:, :])
        nc.sync.dma_start(
            out=ctfl_i[:, :],
            in_=ctp.ap().rearrange("(p t) -> p t", t=LTT))
        nc.vector.tensor_copy(out=ctfl[:, :], in_=ctfl_i[:, :])
        nc.vector.tensor_scalar(out=posml[:, :], in0=ctfl[:, :], scalar1=0.5,
                                scalar2=None, op0=ALU.is_gt)

        # ---- loc DMAs ----
        for name, param in (("T", loc_T), ("S", loc_S), ("t", loc_t)):
            dst = locsb[name]
            flat = param.ap()
            for a in range(2):
                nc.sync.dma_start(
                    out=dst[:, a, 0:1092],
                    in_=flat[:, :]
                        .rearrange("(p a j) f -> p a (j f)", a=2, j=273)[:, a, :])

        # ---- loc smooth-L1 (masked, sum) ----
        # per element: 0.5*min(u,1)^2 + max(u,1) - 1 with u = |loc - loc_t|*pos
        # masked/pad elements contribute exactly +1, subtracted as NE_CONST.
        if STAGE >= 2:
            posml4 = (posml[:, :].rearrange("p (a j) -> p a j", a=2)
                      .unsqueeze(3).broadcast_to((128, 2, 273, 4)))
            for x, col in (("T", COL_LT), ("S", COL_LS)):
                nc.vector.tensor_tensor(
                    out=ld[:, :].rearrange("p (a e) -> p a e", a=2),
                    in0=locsb[x][:, :, 0:1092],
                    in1=locsb["t"][:, :, 0:1092],
                    op=ALU.subtract)
                nc.vector.tensor_tensor(
                    out=lu[:, :].rearrange("p (a j f) -> p a j f", a=2, j=273),
                    in0=ld[:, :].rearrange("p (a j f) -> p a j f", a=2, j=273),
                    in1=posml4, op=ALU.mult)
                nc.scalar.activation(out=lu[:, :], in_=lu[:, :], func=ACT.Abs)
                nc.vector.tensor_scalar(out=lc_[:, :], in0=lu[:, :], scalar1=1.0,
                                        scalar2=None, op0=ALU.min)
                nc.vector.tensor_scalar(out=lm[:, :], in0=lu[:, :], scalar1=1.0,
                                        scalar2=None, op0=ALU.max)
                nc.scalar.activation(out=lc_[:, :], in_=lc_[:, :], func=ACT.Square,
                                     scale=float(1.0 / np.sqrt(2.0)))
                nc.vector.tensor_tensor(out=ld[:, :], in0=lc_[:, :],
                                        in1=lm[:, :], op=ALU.add)
                nc.vector.tensor_reduce(out=partials[:, col:col + 1],
                                        in_=ld[:, :],
                                        axis=mybir.AxisListType.X, op=ALU.add)


        # ---- final partition reduce of partials -> out ----
        psF = psum.tile([1, NPART], F32, name="psF", tag="ps")
        nc.tensor.matmul(psF[:, :], lhsT=ones_sb[:, :], rhs=partials[:, :],
                         start=True, stop=True)
        fin = small.tile([1, NPART], F32)
        nc.vector.tensor_copy(out=fin[:, :], in_=psF[:, :])
        nc.sync.dma_start(out=out_p.ap(), in_=fin[:, :])
    nc.finalize()
    return nc


_NC_CACHE = None


def _get_nc():
    global _NC_CACHE
    if _NC_CACHE is None:
        _NC_CACHE = build_nc()
    return _NC_CACHE


def _host_consts():
    iota = np.ascontiguousarray(
        np.tile(np.arange(C, dtype=np.float32), FBT)[None, :].repeat(128, 0))
    ones8w = np.ones((8, 128), np.float32)
    eye81 = np.eye(81, dtype=np.float32)
    ones = np.ones((128, 1), np.float32)
    return iota, ones8w, eye81, ones


def _build_in_maps(inputs):
    conf_T = np.ascontiguousarray(np.asarray(inputs["conf_dataT"], np.float32))
    conf_S = np.ascontiguousarray(np.asarray(inputs["conf_dataS"], np.float32))
    loc_T = np.ascontiguousarray(np.asarray(inputs["loc_dataT"], np.float32))
    loc_S = np.ascontiguousarray(np.asarray(inputs["loc_dataS"], np.float32))
    loc_t = np.ascontiguousarray(np.asarray(inputs["loc_t"], np.float32))
    ct = np.ascontiguousarray(np.asarray(inputs["conf_t"], np.int32))
    PADN = 128 * LTT - R * P

    def _padloc(a):
        flat = a.reshape(R * P, 4)
        return np.ascontiguousarray(
            np.pad(flat, ((0, PADN), (0, 0))))
    iota, ones8w, eye81, ones = _host_consts()
    in_maps = []
    for d in range(NCORES):
        sl = slice(d * R, (d + 1) * R)
        ctsl = ct[sl]
        in_maps.append({
            "conf_T": conf_T[sl], "conf_S": conf_S[sl],
            "loc_T": _padloc(loc_T[sl]), "loc_S": _padloc(loc_S[sl]),
            "loc_t": _padloc(loc_t[sl]),
            "conf_t": ctsl,
            "ctp": np.ascontiguousarray(
                np.pad(ctsl.ravel(), (0, PADN), constant_values=-1)),
            "iota": iota, "ones8w": ones8w,
            "eye81": eye81, "ones128": ones,
        })
    return in_maps


def _combine(parts):
    S = parts.astype(np.float64).sum(axis=0)
    loss_cT = S[COL_AT] - S[COL_BT] + S[COL_CT] / 2 - S[COL_DT] + S[COL_TKT]
    loss_cS = S[COL_AS] - S[COL_BS] + S[COL_CS] / 2 - S[COL_DS] + S[COL_TKS]
    loss_lT = S[COL_LT] - NCORES * NE_CONST
    loss_lS = S[COL_LS] - NCORES * NE_CONST
    N = S[COL_NP]
    return np.array([loss_lT / N, loss_cT / N, loss_lS / N, loss_cS / N],
                    np.float32)


def run_on_hw(inputs, trace=False, **kw):
    nc = _get_nc()
    in_maps = _build_in_maps(inputs)
    res = run_bass_kernel_spmd(nc, in_maps, core_ids=list(range(NCORES)),
                               trace=trace, **kw)
    parts = np.stack([np.asarray(r["out"]).reshape(NPART) for r in res.results])
    return _combine(parts), res


def kernel(**inputs) -> np.ndarray:
    out, _ = run_on_hw(inputs, trace=False)
    return out


# revision 29
# speedup vs baseline: 1.3363x; 1.1437x over previous
"""Trainium2 Bass kernel for nn_AdaptiveMultiBoxLoss (SSD multibox distillation loss).

Data-parallel over the batch dim across 8 NeuronCores.  Each core computes
partial sums (smooth-L1 loc losses, CE conf losses with hard-negative mining
via a per-row binary-search threshold top-k) over its 8 batch rows; the host
sums the 8x16 partials and performs the final division by N.

Key device-side decompositions:
  loss_c = sum_pos(lse) - sum_all conf[p, ct_p] + sum_all conf[:,0]
           - sum_pos conf[:,0] + topk(lc_masked)
  (exploits that ~98% of priors are background so the CE gather is column 0;
   the true gather sum is a one-hot trace accumulated on the TensorEngine)
  topk per row: binary-search a threshold with exact counts
  (tensor_scalar is_gt + fused accumulate), then an exact correction pass.
"""

import os
import sys

sys.path.insert(0, "/opt/trn_rl_repo")

from contextlib import ExitStack

import numpy as np

import concourse.bass as bass
import concourse.bacc as bacc
import concourse.mybir as mybir
import concourse.tile as tile
from concourse.bass_utils import run_bass_kernel_spmd

F32 = mybir.dt.float32
BF16 = mybir.dt.bfloat16
I32 = mybir.dt.int32
ALU = mybir.AluOpType
ACT = mybir.ActivationFunctionType

# ---- problem geometry (hardcoded) ----
B, P, C = 64, 8732, 81
NCORES = 8
R = B // NCORES            # 8 batch rows per core
NT = 69                    # 128-prior tiles per row (68 full + 1x28)
TFULL, TREM = 68, 28
TCOL = R * NT              # 552 columns in row-tiled layout
NFB, FBT = 3, 23           # conf stream: 3 blocks/row x 23 tiles
FBF = FBT * C              # 1863
LTT, LTFULL, LTREM = 546, 545, 96   # loc flat tiling: 546 tiles of 128 rows
LF = LTT * 4               # 2184
NPART = 16
NE_CONST = 128 * LF        # every element of the padded loc tile contributes +1
NITER = 7                  # binary search iterations (2*lc domain)
HI_INIT = 32.0

# partials columns
(COL_BT, COL_BS, COL_AT, COL_CT, COL_DT, COL_AS, COL_CS, COL_DS,
 COL_LT, COL_LS, COL_TKT, COL_TKS, COL_NP) = range(13)

STAGE = int(os.environ.get("K_STAGE", "9"))


def build_nc():
    nc = bacc.Bacc("TRN2", target_bir_lowering=False, debug=False,
                   num_devices=NCORES)

    conf_T = nc.declare_dram_parameter("conf_T", [R, P, C], F32, isOutput=False)
    conf_S = nc.declare_dram_parameter("conf_S", [R, P, C], F32, isOutput=False)
    loc_T = nc.declare_dram_parameter("loc_T", [128 * LTT, 4], F32, isOutput=False)
    loc_S = nc.declare_dram_parameter("loc_S", [128 * LTT, 4], F32, isOutput=False)
    loc_t = nc.declare_dram_parameter("loc_t", [128 * LTT, 4], F32, isOutput=False)
    ctp = nc.declare_dram_parameter("ctp", [128 * LTT], I32, isOutput=False)
    conf_t = nc.declare_dram_parameter("conf_t", [R, P], I32, isOutput=False)
    iota_p = nc.declare_dram_parameter("iota", [128, FBF], F32, isOutput=False)
    onesw_p = nc.declare_dram_parameter("ones8w", [8, 128], F32, isOutput=False)
    eye_p = nc.declare_dram_parameter("eye81", [81, 81], F32, isOutput=False)
    ones_p = nc.declare_dram_parameter("ones128", [128, 1], F32, isOutput=False)
    out_p = nc.declare_dram_parameter("out", [1, NPART], F32, isOutput=True)

    with tile.TileContext(nc) as tc, ExitStack() as ctx:
        cpool = ctx.enter_context(tc.tile_pool(name="consts", bufs=1))
        pers = ctx.enter_context(tc.tile_pool(name="pers", bufs=1))
        small = ctx.enter_context(tc.tile_pool(name="small", bufs=1))
        pool_cT = ctx.enter_context(tc.tile_pool(name="confT", bufs=3))
        pool_cS = ctx.enter_context(tc.tile_pool(name="confS", bufs=3))
        pool_eT = ctx.enter_context(tc.tile_pool(name="expT", bufs=2))
        pool_eS = ctx.enter_context(tc.tile_pool(name="expS", bufs=2))
        pool_eq = ctx.enter_context(tc.tile_pool(name="eq", bufs=2))
        psum = ctx.enter_context(tc.tile_pool(name="ps", bufs=4, space="PSUM"))
        pstr = ctx.enter_context(tc.tile_pool(name="tr", bufs=1, space="PSUM"))

        # ---- constants ----
        iota_sb = cpool.tile([128, FBF], F32)
        onesw_sb = cpool.tile([8, 128], F32)
        eye_sb = cpool.tile([81, 81], F32)
        ones_sb = cpool.tile([128, 1], F32)
        nc.sync.dma_start(out=iota_sb[:, :], in_=iota_p.ap())
        nc.sync.dma_start(out=onesw_sb[:, :], in_=onesw_p.ap())
        nc.sync.dma_start(out=eye_sb[:, :], in_=eye_p.ap())
        nc.sync.dma_start(out=ones_sb[:, :], in_=ones_p.ap())

        # ---- persistent tensors ----
        ctf_i = pers.tile([128, TCOL], I32)
        ctf = pers.tile([128, TCOL], F32)
        posf = pers.tile([128, TCOL], F32)
        ominus = pers.tile([128, TCOL], F32)
        vmask = pers.tile([128, TCOL], F32)
        sumexp = {x: pers.tile([128, TCOL], F32, name=f"sumexp{x}") for x in "TS"}
        conf0 = {x: pers.tile([128, TCOL], F32, name=f"conf0{x}") for x in "TS"}
        lse = {x: pers.tile([128, TCOL], F32, name=f"lse{x}") for x in "TS"}
        lcm = {x: pers.tile([128, TCOL], F32, name=f"lcm{x}") for x in "TS"}
        partials = pers.tile([128, NPART], F32)
        sgnjunk = pers.tile([128, TCOL], F32)
        sjunk = {x: pers.tile([128, TCOL], F32, name=f"sjunk{x}") for x in "TS"}

        ctfl_i = pers.tile([128, LTT], I32)
        ctfl = pers.tile([128, LTT], F32)
        posml = pers.tile([128, LTT], F32)
        locsb = {n: pers.tile([128, 2, 1096], F32, name=f"loc{n}")
                 for n in ("T", "S", "t")}
        ld = pers.tile([128, LF], F32)
        lu = pers.tile([128, LF], F32)
        lc_ = pers.tile([128, LF], F32)
        lm = pers.tile([128, LF], F32)

        nc.gpsimd.memset(partials[:, :], 0.0)

        # ---- conf_t: row-tiled layout, partition-major within each row ----
        # column r*NT + fb*FBT + j on partition p holds prior 69*p + fb*23 + j
        for r in range(R):
            nc.gpsimd.memset(ctf_i[96:128, r * NT:(r + 1) * NT], -1)
            nc.sync.dma_start(
                out=ctf_i[0:126, r * NT:(r + 1) * NT],
                in_=conf_t.ap()[r, 0:126 * NT].rearrange("(p t) -> p t", t=NT))
            nc.sync.dma_start(
                out=ctf_i[126:127, r * NT:r * NT + 38],
                in_=conf_t.ap()[r, 126 * NT:P].unsqueeze(0))
        nc.vector.tensor_copy(out=ctf[:, :], in_=ctf_i[:, :])
        nc.vector.tensor_scalar(out=posf[:, :], in0=ctf[:, :], scalar1=0.5,
                                scalar2=None, op0=ALU.is_gt)
        nc.vector.tensor_scalar(out=vmask[:, :], in0=ctf[:, :], scalar1=-0.5,
                                scalar2=2.0, op0=ALU.is_gt, op1=ALU.mult)
        nc.vector.scalar_tensor_tensor(out=ominus[:, :], in0=posf[:, :],
                                       scalar=-2.0, in1=vmask[:, :],
                                       op0=ALU.mult, op1=ALU.add)

        # num_pos per row -> k
        npp = small.tile([128, 8], F32)
        nc.vector.tensor_reduce(out=npp[:, :],
                                in_=posf[:, :].rearrange("p (r t) -> p r t", r=R),
                                axis=mybir.AxisListType.X, op=ALU.add)
        ps_np = psum.tile([8, 1], F32, tag="ps")
        nc.tensor.matmul(ps_np[:, :], lhsT=npp[:, :], rhs=ones_sb[:, :],
                         start=True, stop=True)
        np8 = small.tile([8, 1], F32)
        nc.vector.tensor_copy(out=np8[:, :], in_=ps_np[:, :])
        k8 = small.tile([8, 1], F32)
        nc.vector.tensor_scalar(out=k8[:, :], in0=np8[:, :], scalar1=3.0,
                                scalar2=float(P - 1), op0=ALU.mult, op1=ALU.min)
        nc.vector.tensor_copy(out=partials[0:8, COL_NP:COL_NP + 1], in_=np8[:, :])

        # ---- conf streaming loop ----
        pstr_t = {x: pstr.tile([81, 81], F32, name=f"pstr{x}") for x in "TS"}
        if STAGE >= 3:
            nmm = {"T": 0, "S": 0}
            fbidx = [0]
            total_mm = R * NFB * FBT
            for r in range(R):
                for fb in range(NFB):
                    cb = r * NT + fb * FBT
                    pb = fb * FBT * 128
                    ct_fb = {"T": pool_cT.tile([128, FBT, C], F32, name="ctT"),
                             "S": pool_cS.tile([128, FBT, C], F32, name="ctS")}
                    ex_fb = {"T": pool_eT.tile([128, FBT, C], BF16, name="exT"),
                             "S": pool_eS.tile([128, FBT, C], BF16, name="exS")}
                    for x, param in (("T", conf_T), ("S", conf_S)):
                        t = ct_fb[x]
                        if fbidx[0] < 3:
                            # fresh SBUF slot: clear pad partitions once so
                            # later exp() of unwritten pads stays finite
                            nc.scalar.memzero(t[96:128, :, :])
                        # src view: conf[r, 69p + fb*23 + j, c]
                        rowv = param.ap()[r, :, :]
                        main = rowv[0:126 * NT, :].rearrange(
                            "(p t) c -> p t c", t=NT)
                        nc.sync.dma_start(
                            out=t[0:126, :, :],
                            in_=main[:, fb * FBT:(fb + 1) * FBT, :])
                        if fb == 0:
                            nc.sync.dma_start(
                                out=t[126:127, :, :],
                                in_=rowv[126 * NT:126 * NT + FBT, :]
                                    .unsqueeze(0))
                        elif fb == 1:
                            nc.sync.dma_start(
                                out=t[126:127, 0:15, :],
                                in_=rowv[126 * NT + FBT:P, :].unsqueeze(0))
                        nc.scalar.activation(out=ex_fb[x][:, :, :],
                                             in_=t[:, :, 0:C], func=ACT.Exp)

                    eq_t = pool_eq.tile([128, FBT, C], F32, name="eqt")
                    ctb_view = ctf[:, cb:cb + FBT].unsqueeze(2).broadcast_to(
                        (128, FBT, C))
                    nc.vector.tensor_tensor(
                        out=eq_t[:, :, :],
                        in0=iota_sb[:, :].rearrange("p (t c) -> p t c", c=C),
                        in1=ctb_view, op=ALU.is_equal)

                    for x in "TS":
                        e1 = pool_eq.tile([128, FBT, 40], BF16, name="e1",
                                          tag="e1")
                        nc.vector.tensor_tensor(out=e1[:, :, :],
                                                in0=ex_fb[x][:, :, 0:40],
                                                in1=ex_fb[x][:, :, 40:80],
                                                op=ALU.add)
                        nc.vector.tensor_reduce(out=sumexp[x][:, cb:cb + FBT],
                                                in_=e1[:, :, :],
                                                axis=mybir.AxisListType.X,
                                                op=ALU.add)
                        nc.vector.tensor_tensor(out=sumexp[x][:, cb:cb + FBT],
                                                in0=sumexp[x][:, cb:cb + FBT],
                                                in1=ex_fb[x][:, :, 80],
                                                op=ALU.add)
                        nc.vector.tensor_copy(out=conf0[x][:, cb:cb + FBT],
                                              in_=ct_fb[x][:, :, 0])
                        # one-hot trace: psum[m, c] += sum_p eq[p, m]*conf[p, c]
                        for t in range(FBT):
                            nc.tensor.matmul(pstr_t[x][:, :],
                                             lhsT=eq_t[:, t, :],
                                             rhs=ct_fb[x][:, t, 0:C],
                                             start=(nmm[x] == 0),
                                             stop=(nmm[x] == total_mm - 1))
                            nmm[x] += 1
                fbidx[0] += 1
                # per-row tail once the row's 3 blocks are done
                if fb == NFB - 1 and STAGE >= 4:
                    rc = r * NT
                    for x in "TS":
                        nc.scalar.activation(out=lse[x][:, rc:rc + NT],
                                             in_=sumexp[x][:, rc:rc + NT],
                                             func=ACT.Ln)
                        nc.vector.scalar_tensor_tensor(
                            out=sumexp[x][:, rc:rc + NT],
                            in0=conf0[x][:, rc:rc + NT], scalar=-1.0,
                            in1=lse[x][:, rc:rc + NT],
                            op0=ALU.mult, op1=ALU.add)
                        nc.vector.tensor_tensor(out=lcm[x][:, rc:rc + NT],
                                                in0=sumexp[x][:, rc:rc + NT],
                                                in1=ominus[:, rc:rc + NT],
                                                op=ALU.mult)

        # ---- per-tensor epilogue: lse, partial sums, lc_m ----
        if STAGE >= 4:
            for x, (colA, colCc, colD, colB) in (
                    ("T", (COL_AT, COL_CT, COL_DT, COL_BT)),
                    ("S", (COL_AS, COL_CS, COL_DS, COL_BS))):
                # A = sum(lse * posf)
                nc.vector.tensor_tensor(out=sgnjunk[:, :], in0=lse[x][:, :],
                                        in1=posf[:, :], op=ALU.mult)
                nc.vector.tensor_reduce(out=partials[:, colA:colA + 1],
                                        in_=sgnjunk[:, :],
                                        axis=mybir.AxisListType.X, op=ALU.add)
                # C2 = sum conf0 * vmask2  (= 2*C, pads excluded)
                nc.vector.tensor_tensor(out=sgnjunk[:, :], in0=conf0[x][:, :],
                                        in1=vmask[:, :], op=ALU.mult)
                nc.vector.tensor_reduce(out=partials[:, colCc:colCc + 1],
                                        in_=sgnjunk[:, :],
                                        axis=mybir.AxisListType.X, op=ALU.add)
                # D = sum conf0 * posf
                nc.vector.tensor_tensor(out=sgnjunk[:, :], in0=conf0[x][:, :],
                                        in1=posf[:, :], op=ALU.mult)
                nc.vector.tensor_reduce(out=partials[:, colD:colD + 1],
                                        in_=sgnjunk[:, :],
                                        axis=mybir.AxisListType.X, op=ALU.add)
                # B = trace(pstr): diag via eye mask
                nc.vector.tensor_tensor(out=sgnjunk[0:81, 0:81],
                                        in0=pstr_t[x][:, :], in1=eye_sb[:, :],
                                        op=ALU.mult)
                nc.vector.tensor_reduce(out=partials[0:81, colB:colB + 1],
                                        in_=sgnjunk[0:81, 0:81],
                                        axis=mybir.AxisListType.X, op=ALU.add)

        # ---- binary search for per-row top-k count thresholds ----
        # natural layout: per-row thresholds broadcast to [128, 8] via
        # diag(t) matmul, counts via 8 per-row tensor_scalar+accum ops.
        lo = {x: small.tile([8, 1], F32, name=f"lo{x}") for x in "TS"}
        hi = {x: small.tile([8, 1], F32, name=f"hi{x}") for x in "TS"}
        tmid = {x: small.tile([8, 1], F32, name=f"tm{x}") for x in "TS"}
        ge = {x: small.tile([8, 1], I32, name=f"ge{x}") for x in "TS"}
        gei = {x: small.tile([8, 1], I32, name=f"gei{x}") for x in "TS"}
        s8 = {x: small.tile([8, 1], F32, name=f"s8{x}") for x in "TS"}
        diag8 = {x: small.tile([8, 8], F32, name=f"dg{x}") for x in "TS"}
        trep = {x: small.tile([128, 8], F32, name=f"trep{x}") for x in "TS"}
        cnt8 = {x: small.tile([128, 8], F32, name=f"cnt8{x}") for x in "TS"}
        s8p = {x: small.tile([128, 8], F32, name=f"s8p{x}") for x in "TS"}
        ns8 = {x: small.tile([8, 2], F32, name=f"ns8{x}") for x in "TS"}
        tk = {x: small.tile([8, 1], F32, name=f"tk{x}") for x in "TS"}

        def bcast_rows(vec8, x):
            # trep[q, r] = vec8[r]  for all partitions q
            nc.vector.tensor_tensor(out=diag8[x][:, :], in0=eye_sb[0:8, 0:8],
                                    in1=vec8[:, :].broadcast_to((8, 8)),
                                    op=ALU.mult)
            psA = psum.tile([128, 8], F32, name="psA", tag="ps")
            nc.tensor.matmul(psA[:, :], lhsT=onesw_sb[:, :],
                             rhs=diag8[x][:, :], start=True, stop=True)
            nc.vector.tensor_copy(out=trep[x][:, :], in_=psA[:, :])

        def row_counts(x, src_tile, out128x8):
            trv = trep[x][:, :].unsqueeze(2).broadcast_to((128, R, NT))
            nc.vector.tensor_tensor(
                out=sjunk[x][:, :].rearrange("p (r t) -> p r t", r=R),
                in0=src_tile[:, :].rearrange("p (r t) -> p r t", r=R),
                in1=trv, op=ALU.is_gt)
            nc.vector.tensor_reduce(
                out=out128x8[:, :],
                in_=sjunk[x][:, :].rearrange("p (r t) -> p r t", r=R),
                axis=mybir.AxisListType.X, op=ALU.add)

        if STAGE >= 5:
            for x in "TS":
                nc.gpsimd.memset(lo[x][:, :], 0.0)
                nc.gpsimd.memset(hi[x][:, :], HI_INIT)
            for it in range(NITER):
                for x in "TS":
                    nc.vector.tensor_tensor(out=tmid[x][:, :], in0=lo[x][:, :],
                                            in1=hi[x][:, :], op=ALU.add)
                    nc.vector.tensor_scalar(out=tmid[x][:, :], in0=tmid[x][:, :],
                                            scalar1=0.5, scalar2=None,
                                            op0=ALU.mult)
                    bcast_rows(tmid[x], x)
                    row_counts(x, lcm[x], cnt8[x])
                    psB = psum.tile([8, 1], F32, name="psB", tag="ps")
                    nc.tensor.matmul(psB[:, :], lhsT=cnt8[x][:, :],
                                     rhs=ones_sb[:, :], start=True, stop=True)
                    nc.vector.tensor_copy(out=s8[x][:, :], in_=psB[:, :])
                    nc.vector.tensor_tensor(out=ge[x][:, :], in0=s8[x][:, :],
                                            in1=k8[:, :], op=ALU.is_ge)
                    nc.vector.copy_predicated(out=lo[x][:, :], mask=ge[x][:, :],
                                              data=tmid[x][:, :])
                    nc.vector.tensor_scalar(out=gei[x][:, :], in0=ge[x][:, :],
                                            scalar1=1, scalar2=None,
                                            op0=ALU.bitwise_xor)
                    nc.vector.copy_predicated(out=hi[x][:, :], mask=gei[x][:, :],
                                              data=tmid[x][:, :])

        # ---- exact pass at t* = lo ----
        if STAGE >= 6:
            for x, colk in (("T", COL_TKT), ("S", COL_TKS)):
                bcast_rows(lo[x], x)
                row_counts(x, lcm[x], cnt8[x])
                nc.vector.tensor_tensor(out=lse[x][:, :], in0=lcm[x][:, :],
                                        in1=sjunk[x][:, :], op=ALU.mult)
                nc.vector.tensor_reduce(
                    out=s8p[x][:, :],
                    in_=lse[x][:, :].rearrange("p (r t) -> p r t", r=R),
                    axis=mybir.AxisListType.X, op=ALU.add)
                psC = psum.tile([8, 2], F32, name="psC", tag="ps")
                nc.tensor.matmul(psC[:, 0:1], lhsT=cnt8[x][:, :],
                                 rhs=ones_sb[:, :], start=True, stop=True)
                nc.tensor.matmul(psC[:, 1:2], lhsT=s8p[x][:, :],
                                 rhs=ones_sb[:, :], start=True, stop=True)
                nc.vector.tensor_copy(out=ns8[x][:, :], in_=psC[:, :])
                # topk = S* + (k - n*) * t*
                nc.vector.tensor_tensor(out=tk[x][:, :], in0=k8[:, :],
                                        in1=ns8[x][:, 0:1], op=ALU.subtract)
                nc.vector.tensor_tensor(out=tk[x][:, :], in0=tk[x][:, :],
                                        in1=lo[x][:, :], op=ALU.mult)
                nc.vector.tensor_tensor(out=tk[x][:, :], in0=tk[x][:, :],
                                        in1=ns8[x][:, 1:2], op=ALU.add)
                nc.vector.tensor_scalar(out=tk[x][:, :], in0=tk[x][:, :],
                                        scalar1=0.5, scalar2=None, op0=ALU.mult)
                nc.vector.tensor_copy(out=partials[0:8, colk:colk + 1],
                                      in_=tk[x][:, :])

        # ---- conf_t flat layout (for loc masking) ----
        # gates: loc DMAs wait for end-of-streaming; the loc vector chain
        # waits for the search to finish so it fills the tail instead of
        # stalling the vector FIFO mid-kernel.
        if STAGE >= 6:
            for n in ("T", "S", "t"):
                nc.vector.tensor_copy(out=locsb[n][0:8, 0, 0:1],
                                      in_=lcm["S"][0:8, TCOL - 1:TCOL])
            nc.vector.tensor_copy(out=ld[0:8, 0:1], in_=tk["S"][:, :])
        nc.sync.dma_start(
            out=ctfl_i[:, :],
            in_=ctp.ap().rearrange("(p t) -> p t", t=LTT))
        nc.vector.tensor_copy(out=ctfl[:, :], in_=ctfl_i[:, :])
        nc.vector.tensor_scalar(out=posml[:, :], in0=ctfl[:, :], scalar1=0.5,
                                scalar2=None, op0=ALU.is_gt)

        # ---- loc DMAs ----
        for name, param in (("T", loc_T), ("S", loc_S), ("t", loc_t)):
            dst = locsb[name]
            flat = param.ap()
            for a in range(2):
                nc.sync.dma_start(
                    out=dst[:, a, 0:1092],
                    in_=flat[:, :]
                        .rearrange("(p a j) f -> p a (j f)", a=2, j=273)[:, a, :])

        # ---- loc smooth-L1 (masked, sum) ----
        # per element: 0.5*min(u,1)^2 + max(u,1) - 1 with u = |loc - loc_t|*pos
        # masked/pad elements contribute exactly +1, subtracted as NE_CONST.
        if STAGE >= 2:
            posml4 = (posml[:, :].rearrange("p (a j) -> p a j", a=2)
                      .unsqueeze(3).broadcast_to((128, 2, 273, 4)))
            for x, col in (("T", COL_LT), ("S", COL_LS)):
                nc.vector.tensor_tensor(
                    out=ld[:, :].rearrange("p (a e) -> p a e", a=2),
                    in0=locsb[x][:, :, 0:1092],
                    in1=locsb["t"][:, :, 0:1092],
                    op=ALU.subtract)
                nc.vector.tensor_tensor(
                    out=lu[:, :].rearrange("p (a j f) -> p a j f", a=2, j=273),
                    in0=ld[:, :].rearrange("p (a j f) -> p a j f", a=2, j=273),
                    in1=posml4, op=ALU.mult)
                nc.scalar.activation(out=lu[:, :], in_=lu[:, :], func=ACT.Abs)
                nc.vector.tensor_scalar(out=lc_[:, :], in0=lu[:, :], scalar1=1.0,
                                        scalar2=None, op0=ALU.min)
                nc.vector.tensor_scalar(out=lm[:, :], in0=lu[:, :], scalar1=1.0,
                                        scalar2=None, op0=ALU.max)
                nc.scalar.activation(out=lc_[:, :], in_=lc_[:, :], func=ACT.Square,
                                     scale=float(1.0 / np.sqrt(2.0)))
                nc.vector.tensor_tensor(out=ld[:, :], in0=lc_[:, :],
                                        in1=lm[:, :], op=ALU.add)
                nc.vector.tensor_reduce(out=partials[:, col:col + 1],
                                        in_=ld[:, :],
                                        axis=mybir.AxisListType.X, op=ALU.add)


        # ---- final partition reduce of partials -> out ----
        psF = psum.tile([1, NPART], F32, name="psF", tag="ps")
        nc.tensor.matmul(psF[:, :], lhsT=ones_sb[:, :], rhs=partials[:, :],
                         start=True, stop=True)
        fin = small.tile([1, NPART], F32)
        nc.vector.tensor_copy(out=fin[:, :], in_=psF[:, :])
        nc.sync.dma_start(out=out_p.ap(), in_=fin[:, :])
    nc.finalize()
    return nc


_NC_CACHE = None


def _get_nc():
    global _NC_CACHE
    if _NC_CACHE is None:
        _NC_CACHE = build_nc()
    return _NC_CACHE


def _host_consts():
    iota = np.ascontiguousarray(
        np.tile(np.arange(C, dtype=np.float32), FBT)[None, :].repeat(128, 0))
    ones8w = np.ones((8, 128), np.float32)
    eye81 = np.eye(81, dtype=np.float32)
    ones = np.ones((128, 1), np.float32)
    return iota, ones8w, eye81, ones


def _build_in_maps(inputs):
    conf_T = np.ascontiguousarray(np.asarray(inputs["conf_dataT"], np.float32))
    conf_S = np.ascontiguousarray(np.asarray(inputs["conf_dataS"], np.float32))
    loc_T = np.ascontiguousarray(np.asarray(inputs["loc_dataT"], np.float32))
    loc_S = np.ascontiguousarray(np.asarray(inputs["loc_dataS"], np.float32))
    loc_t = np.ascontiguousarray(np.asarray(inputs["loc_t"], np.float32))
    ct = np.ascontiguousarray(np.asarray(inputs["conf_t"], np.int32))
    PADN = 128 * LTT - R * P

    def _padloc(a):
        flat = a.reshape(R * P, 4)
        return np.ascontiguousarray(
            np.pad(flat, ((0, PADN), (0, 0))))
    iota, ones8w, eye81, ones = _host_consts()
    in_maps = []
    for d in range(NCORES):
        sl = slice(d * R, (d + 1) * R)
        ctsl = ct[sl]
        in_maps.append({
            "conf_T": conf_T[sl], "conf_S": conf_S[sl],
            "loc_T": _padloc(loc_T[sl]), "loc_S": _padloc(loc_S[sl]),
            "loc_t": _padloc(loc_t[sl]),
            "conf_t": ctsl,
            "ctp": np.ascontiguousarray(
                np.pad(ctsl.ravel(), (0, PADN), constant_values=-1)),
            "iota": iota, "ones8w": ones8w,
            "eye81": eye81, "ones128": ones,
        })
    return in_maps


def _combine(parts):
    S = parts.astype(np.float64).sum(axis=0)
    loss_cT = S[COL_AT] - S[COL_BT] + S[COL_CT] / 2 - S[COL_DT] + S[COL_TKT]
    loss_cS = S[COL_AS] - S[COL_BS] + S[COL_CS] / 2 - S[COL_DS] + S[COL_TKS]
    loss_lT = S[COL_LT] - NCORES * NE_CONST
    loss_lS = S[COL_LS] - NCORES * NE_CONST
    N = S[COL_NP]
    return np.array([loss_lT / N, loss_cT / N, loss_lS / N, loss_cS / N],
                    np.float32)


def run_on_hw(inputs, trace=False, **kw):
    nc = _get_nc()
    in_maps = _build_in_maps(inputs)
    res = run_bass_kernel_spmd(nc, in_maps, core_ids=list(range(NCORES)),
                               trace=trace, **kw)
    parts = np.stack([np.asarray(r["out"]).reshape(NPART) for r in res.results])
    return _combine(parts), res


def kernel(**inputs) -> np.ndarray:
    out, _ = run_on_hw(inputs, trace=False)
    return out


# revision 30
# speedup vs baseline: 1.3508x; 1.0108x over previous
"""Trainium2 Bass kernel for nn_AdaptiveMultiBoxLoss (SSD multibox distillation loss).

Data-parallel over the batch dim across 8 NeuronCores.  Each core computes
partial sums (smooth-L1 loc losses, CE conf losses with hard-negative mining
via a per-row binary-search threshold top-k) over its 8 batch rows; the host
sums the 8x16 partials and performs the final division by N.

Key device-side decompositions:
  loss_c = sum_pos(lse) - sum_all conf[p, ct_p] + sum_all conf[:,0]
           - sum_pos conf[:,0] + topk(lc_masked)
  (exploits that ~98% of priors are background so the CE gather is column 0;
   the true gather sum is a one-hot trace accumulated on the TensorEngine)
  topk per row: binary-search a threshold with exact counts
  (tensor_scalar is_gt + fused accumulate), then an exact correction pass.
"""

import os
import sys

sys.path.insert(0, "/opt/trn_rl_repo")

from contextlib import ExitStack

import numpy as np

import concourse.bass as bass
import concourse.bacc as bacc
import concourse.mybir as mybir
import concourse.tile as tile
from concourse.bass_utils import run_bass_kernel_spmd

F32 = mybir.dt.float32
BF16 = mybir.dt.bfloat16
I32 = mybir.dt.int32
ALU = mybir.AluOpType
ACT = mybir.ActivationFunctionType

# ---- problem geometry (hardcoded) ----
B, P, C = 64, 8732, 81
NCORES = 8
R = B // NCORES            # 8 batch rows per core
NT = 69                    # 128-prior tiles per row (68 full + 1x28)
TFULL, TREM = 68, 28
TCOL = R * NT              # 552 columns in row-tiled layout
NFB, FBT = 3, 23           # conf stream: 3 blocks/row x 23 tiles
FBF = FBT * C              # 1863
LTT, LTFULL, LTREM = 546, 545, 96   # loc flat tiling: 546 tiles of 128 rows
LF = LTT * 4               # 2184
NPART = 16
NE_CONST = 128 * LF        # every element of the padded loc tile contributes +1
NITER = 6                  # binary search iterations (2*lc domain)
HI_INIT = 32.0

# partials columns
(COL_BT, COL_BS, COL_AT, COL_CT, COL_DT, COL_AS, COL_CS, COL_DS,
 COL_LT, COL_LS, COL_TKT, COL_TKS, COL_NP) = range(13)

STAGE = int(os.environ.get("K_STAGE", "9"))


def build_nc():
    nc = bacc.Bacc("TRN2", target_bir_lowering=False, debug=False,
                   num_devices=NCORES)

    conf_T = nc.declare_dram_parameter("conf_T", [R, P, C], F32, isOutput=False)
    conf_S = nc.declare_dram_parameter("conf_S", [R, P, C], F32, isOutput=False)
    loc_T = nc.declare_dram_parameter("loc_T", [128 * LTT, 4], F32, isOutput=False)
    loc_S = nc.declare_dram_parameter("loc_S", [128 * LTT, 4], F32, isOutput=False)
    loc_t = nc.declare_dram_parameter("loc_t", [128 * LTT, 4], F32, isOutput=False)
    ctp = nc.declare_dram_parameter("ctp", [128 * LTT], I32, isOutput=False)
    conf_t = nc.declare_dram_parameter("conf_t", [R, P], I32, isOutput=False)
    iota_p = nc.declare_dram_parameter("iota", [128, FBF], F32, isOutput=False)
    onesw_p = nc.declare_dram_parameter("ones8w", [8, 128], F32, isOutput=False)
    eye_p = nc.declare_dram_parameter("eye81", [81, 81], F32, isOutput=False)
    ones_p = nc.declare_dram_parameter("ones128", [128, 1], F32, isOutput=False)
    out_p = nc.declare_dram_parameter("out", [1, NPART], F32, isOutput=True)

    with tile.TileContext(nc) as tc, ExitStack() as ctx:
        cpool = ctx.enter_context(tc.tile_pool(name="consts", bufs=1))
        pers = ctx.enter_context(tc.tile_pool(name="pers", bufs=1))
        small = ctx.enter_context(tc.tile_pool(name="small", bufs=1))
        pool_cT = ctx.enter_context(tc.tile_pool(name="confT", bufs=3))
        pool_cS = ctx.enter_context(tc.tile_pool(name="confS", bufs=3))
        pool_eT = ctx.enter_context(tc.tile_pool(name="expT", bufs=3))
        pool_eS = ctx.enter_context(tc.tile_pool(name="expS", bufs=3))
        pool_eq = ctx.enter_context(tc.tile_pool(name="eq", bufs=3))
        psum = ctx.enter_context(tc.tile_pool(name="ps", bufs=4, space="PSUM"))
        pstr = ctx.enter_context(tc.tile_pool(name="tr", bufs=1, space="PSUM"))

        # ---- constants ----
        iota_sb = cpool.tile([128, FBF], F32)
        onesw_sb = cpool.tile([8, 128], F32)
        eye_sb = cpool.tile([81, 81], F32)
        ones_sb = cpool.tile([128, 1], F32)
        nc.sync.dma_start(out=iota_sb[:, :], in_=iota_p.ap())
        nc.sync.dma_start(out=onesw_sb[:, :], in_=onesw_p.ap())
        nc.sync.dma_start(out=eye_sb[:, :], in_=eye_p.ap())
        nc.sync.dma_start(out=ones_sb[:, :], in_=ones_p.ap())

        # ---- persistent tensors ----
        ctf_i = pers.tile([128, TCOL], I32)
        ctf = pers.tile([128, TCOL], F32)
        posf = pers.tile([128, TCOL], F32)
        ominus = pers.tile([128, TCOL], F32)
        vmask = pers.tile([128, TCOL], F32)
        sumexp = {x: pers.tile([128, TCOL], F32, name=f"sumexp{x}") for x in "TS"}
        conf0 = {x: pers.tile([128, TCOL], F32, name=f"conf0{x}") for x in "TS"}
        lse = {x: pers.tile([128, TCOL], F32, name=f"lse{x}") for x in "TS"}
        lcm = {x: pers.tile([128, TCOL], F32, name=f"lcm{x}") for x in "TS"}
        partials = pers.tile([128, NPART], F32)
        sgnjunk = pers.tile([128, TCOL], F32)
        sjunk = {x: pers.tile([128, TCOL], F32, name=f"sjunk{x}") for x in "TS"}

        ctfl_i = pers.tile([128, LTT], I32)
        ctfl = pers.tile([128, LTT], F32)
        posml = pers.tile([128, LTT], F32)
        locsb = {n: pers.tile([128, 2, 1096], F32, name=f"loc{n}")
                 for n in ("T", "S", "t")}
        ld = pers.tile([128, LF], F32)
        lu = pers.tile([128, LF], BF16)
        lc_ = pers.tile([128, LF], BF16)
        lm = pers.tile([128, LF], BF16)

        nc.gpsimd.memset(partials[:, :], 0.0)

        # ---- conf_t: row-tiled layout, partition-major within each row ----
        # column r*NT + fb*FBT + j on partition p holds prior 69*p + fb*23 + j
        for r in range(R):
            nc.gpsimd.memset(ctf_i[96:128, r * NT:(r + 1) * NT], -1)
            nc.sync.dma_start(
                out=ctf_i[0:126, r * NT:(r + 1) * NT],
                in_=conf_t.ap()[r, 0:126 * NT].rearrange("(p t) -> p t", t=NT))
            nc.sync.dma_start(
                out=ctf_i[126:127, r * NT:r * NT + 38],
                in_=conf_t.ap()[r, 126 * NT:P].unsqueeze(0))
        nc.vector.tensor_copy(out=ctf[:, :], in_=ctf_i[:, :])
        nc.vector.tensor_scalar(out=posf[:, :], in0=ctf[:, :], scalar1=0.5,
                                scalar2=None, op0=ALU.is_gt)
        nc.vector.tensor_scalar(out=vmask[:, :], in0=ctf[:, :], scalar1=-0.5,
                                scalar2=2.0, op0=ALU.is_gt, op1=ALU.mult)
        nc.vector.scalar_tensor_tensor(out=ominus[:, :], in0=posf[:, :],
                                       scalar=-2.0, in1=vmask[:, :],
                                       op0=ALU.mult, op1=ALU.add)

        # num_pos per row -> k
        npp = small.tile([128, 8], F32)
        nc.vector.tensor_reduce(out=npp[:, :],
                                in_=posf[:, :].rearrange("p (r t) -> p r t", r=R),
                                axis=mybir.AxisListType.X, op=ALU.add)
        ps_np = psum.tile([8, 1], F32, tag="ps")
        nc.tensor.matmul(ps_np[:, :], lhsT=npp[:, :], rhs=ones_sb[:, :],
                         start=True, stop=True)
        np8 = small.tile([8, 1], F32)
        nc.vector.tensor_copy(out=np8[:, :], in_=ps_np[:, :])
        k8 = small.tile([8, 1], F32)
        nc.vector.tensor_scalar(out=k8[:, :], in0=np8[:, :], scalar1=3.0,
                                scalar2=float(P - 1), op0=ALU.mult, op1=ALU.min)
        nc.vector.tensor_copy(out=partials[0:8, COL_NP:COL_NP + 1], in_=np8[:, :])

        # ---- conf streaming loop ----
        pstr_t = {x: pstr.tile([81, 81], F32, name=f"pstr{x}") for x in "TS"}
        if STAGE >= 3:
            nmm = {"T": 0, "S": 0}
            fbidx = [0]
            total_mm = R * NFB * FBT
            for r in range(R):
                for fb in range(NFB):
                    cb = r * NT + fb * FBT
                    pb = fb * FBT * 128
                    ct_fb = {"T": pool_cT.tile([128, FBT, C], F32, name="ctT"),
                             "S": pool_cS.tile([128, FBT, C], F32, name="ctS")}
                    ex_fb = {"T": pool_eT.tile([128, FBT, C], BF16, name="exT"),
                             "S": pool_eS.tile([128, FBT, C], BF16, name="exS")}
                    for x, param in (("T", conf_T), ("S", conf_S)):
                        t = ct_fb[x]
                        if fbidx[0] < 3:
                            # fresh SBUF slot: clear pad partitions once so
                            # later exp() of unwritten pads stays finite
                            nc.scalar.memzero(t[96:128, :, :])
                        # src view: conf[r, 69p + fb*23 + j, c]
                        rowv = param.ap()[r, :, :]
                        main = rowv[0:126 * NT, :].rearrange(
                            "(p t) c -> p t c", t=NT)
                        nc.sync.dma_start(
                            out=t[0:126, :, :],
                            in_=main[:, fb * FBT:(fb + 1) * FBT, :])
                        if fb == 0:
                            nc.sync.dma_start(
                                out=t[126:127, :, :],
                                in_=rowv[126 * NT:126 * NT + FBT, :]
                                    .unsqueeze(0))
                        elif fb == 1:
                            nc.sync.dma_start(
                                out=t[126:127, 0:15, :],
                                in_=rowv[126 * NT + FBT:P, :].unsqueeze(0))
                        nc.scalar.activation(out=ex_fb[x][:, :, :],
                                             in_=t[:, :, 0:C], func=ACT.Exp)

                    eq_t = pool_eq.tile([128, FBT, C], F32, name="eqt")
                    ctb_view = ctf[:, cb:cb + FBT].unsqueeze(2).broadcast_to(
                        (128, FBT, C))
                    nc.vector.tensor_tensor(
                        out=eq_t[:, :, :],
                        in0=iota_sb[:, :].rearrange("p (t c) -> p t c", c=C),
                        in1=ctb_view, op=ALU.is_equal)

                    for x in "TS":
                        e1 = pool_eq.tile([128, FBT, 40], BF16, name="e1",
                                          tag="e1")
                        nc.vector.tensor_tensor(out=e1[:, :, :],
                                                in0=ex_fb[x][:, :, 0:40],
                                                in1=ex_fb[x][:, :, 40:80],
                                                op=ALU.add)
                        nc.vector.tensor_reduce(out=sumexp[x][:, cb:cb + FBT],
                                                in_=e1[:, :, :],
                                                axis=mybir.AxisListType.X,
                                                op=ALU.add)
                        nc.vector.tensor_tensor(out=sumexp[x][:, cb:cb + FBT],
                                                in0=sumexp[x][:, cb:cb + FBT],
                                                in1=ex_fb[x][:, :, 80],
                                                op=ALU.add)
                        nc.scalar.copy(out=conf0[x][:, cb:cb + FBT],
                                       in_=ct_fb[x][:, :, 0])
                        # one-hot trace: psum[m, c] += sum_p eq[p, m]*conf[p, c]
                        for t in range(FBT):
                            nc.tensor.matmul(pstr_t[x][:, :],
                                             lhsT=eq_t[:, t, :],
                                             rhs=ct_fb[x][:, t, 0:C],
                                             start=(nmm[x] == 0),
                                             stop=(nmm[x] == total_mm - 1))
                            nmm[x] += 1
                fbidx[0] += 1
                # per-row tail once the row's 3 blocks are done
                if fb == NFB - 1 and STAGE >= 4:
                    rc = r * NT
                    for x in "TS":
                        nc.scalar.activation(out=lse[x][:, rc:rc + NT],
                                             in_=sumexp[x][:, rc:rc + NT],
                                             func=ACT.Ln)
                        nc.vector.scalar_tensor_tensor(
                            out=sumexp[x][:, rc:rc + NT],
                            in0=conf0[x][:, rc:rc + NT], scalar=-1.0,
                            in1=lse[x][:, rc:rc + NT],
                            op0=ALU.mult, op1=ALU.add)
                        nc.vector.tensor_tensor(out=lcm[x][:, rc:rc + NT],
                                                in0=sumexp[x][:, rc:rc + NT],
                                                in1=ominus[:, rc:rc + NT],
                                                op=ALU.mult)

        # ---- per-tensor epilogue: lse, partial sums, lc_m ----
        if STAGE >= 4:
            for x, (colA, colCc, colD, colB) in (
                    ("T", (COL_AT, COL_CT, COL_DT, COL_BT)),
                    ("S", (COL_AS, COL_CS, COL_DS, COL_BS))):
                # A = sum(lse * posf)
                nc.vector.tensor_tensor(out=sgnjunk[:, :], in0=lse[x][:, :],
                                        in1=posf[:, :], op=ALU.mult)
                nc.vector.tensor_reduce(out=partials[:, colA:colA + 1],
                                        in_=sgnjunk[:, :],
                                        axis=mybir.AxisListType.X, op=ALU.add)
                # C2 = sum conf0 * vmask2  (= 2*C, pads excluded)
                nc.vector.tensor_tensor(out=sgnjunk[:, :], in0=conf0[x][:, :],
                                        in1=vmask[:, :], op=ALU.mult)
                nc.vector.tensor_reduce(out=partials[:, colCc:colCc + 1],
                                        in_=sgnjunk[:, :],
                                        axis=mybir.AxisListType.X, op=ALU.add)
                # D = sum conf0 * posf
                nc.vector.tensor_tensor(out=sgnjunk[:, :], in0=conf0[x][:, :],
                                        in1=posf[:, :], op=ALU.mult)
                nc.vector.tensor_reduce(out=partials[:, colD:colD + 1],
                                        in_=sgnjunk[:, :],
                                        axis=mybir.AxisListType.X, op=ALU.add)
                # B = trace(pstr): diag via eye mask
                nc.vector.tensor_tensor(out=sgnjunk[0:81, 0:81],
                                        in0=pstr_t[x][:, :], in1=eye_sb[:, :],
                                        op=ALU.mult)
                nc.vector.tensor_reduce(out=partials[0:81, colB:colB + 1],
                                        in_=sgnjunk[0:81, 0:81],
                                        axis=mybir.AxisListType.X, op=ALU.add)

        # ---- binary search for per-row top-k count thresholds ----
        # natural layout: per-row thresholds broadcast to [128, 8] via
        # diag(t) matmul, counts via 8 per-row tensor_scalar+accum ops.
        lo = {x: small.tile([8, 1], F32, name=f"lo{x}") for x in "TS"}
        hi = {x: small.tile([8, 1], F32, name=f"hi{x}") for x in "TS"}
        tmid = {x: small.tile([8, 1], F32, name=f"tm{x}") for x in "TS"}
        ge = {x: small.tile([8, 1], I32, name=f"ge{x}") for x in "TS"}
        gei = {x: small.tile([8, 1], I32, name=f"gei{x}") for x in "TS"}
        s8 = {x: small.tile([8, 1], F32, name=f"s8{x}") for x in "TS"}
        diag8 = {x: small.tile([8, 8], F32, name=f"dg{x}") for x in "TS"}
        trep = {x: small.tile([128, 8], F32, name=f"trep{x}") for x in "TS"}
        cnt8 = {x: small.tile([128, 8], F32, name=f"cnt8{x}") for x in "TS"}
        s8p = {x: small.tile([128, 8], F32, name=f"s8p{x}") for x in "TS"}
        ns8 = {x: small.tile([8, 2], F32, name=f"ns8{x}") for x in "TS"}
        tk = {x: small.tile([8, 1], F32, name=f"tk{x}") for x in "TS"}

        def bcast_rows(vec8, x):
            # trep[q, r] = vec8[r]  for all partitions q
            nc.vector.tensor_tensor(out=diag8[x][:, :], in0=eye_sb[0:8, 0:8],
                                    in1=vec8[:, :].broadcast_to((8, 8)),
                                    op=ALU.mult)
            psA = psum.tile([128, 8], F32, name="psA", tag="ps")
            nc.tensor.matmul(psA[:, :], lhsT=onesw_sb[:, :],
                             rhs=diag8[x][:, :], start=True, stop=True)
            nc.vector.tensor_copy(out=trep[x][:, :], in_=psA[:, :])

        def row_counts(x, src_tile, out128x8):
            trv = trep[x][:, :].unsqueeze(2).broadcast_to((128, R, NT))
            nc.vector.tensor_tensor(
                out=sjunk[x][:, :].rearrange("p (r t) -> p r t", r=R),
                in0=src_tile[:, :].rearrange("p (r t) -> p r t", r=R),
                in1=trv, op=ALU.is_gt)
            nc.vector.tensor_reduce(
                out=out128x8[:, :],
                in_=sjunk[x][:, :].rearrange("p (r t) -> p r t", r=R),
                axis=mybir.AxisListType.X, op=ALU.add)

        if STAGE >= 5:
            for x in "TS":
                nc.gpsimd.memset(lo[x][:, :], 0.0)
                nc.gpsimd.memset(hi[x][:, :], HI_INIT)
            for it in range(NITER):
                for x in "TS":
                    nc.vector.tensor_tensor(out=tmid[x][:, :], in0=lo[x][:, :],
                                            in1=hi[x][:, :], op=ALU.add)
                    nc.vector.tensor_scalar(out=tmid[x][:, :], in0=tmid[x][:, :],
                                            scalar1=0.5, scalar2=None,
                                            op0=ALU.mult)
                    bcast_rows(tmid[x], x)
                    row_counts(x, lcm[x], cnt8[x])
                    psB = psum.tile([8, 1], F32, name="psB", tag="ps")
                    nc.tensor.matmul(psB[:, :], lhsT=cnt8[x][:, :],
                                     rhs=ones_sb[:, :], start=True, stop=True)
                    nc.vector.tensor_copy(out=s8[x][:, :], in_=psB[:, :])
                    nc.vector.tensor_tensor(out=ge[x][:, :], in0=s8[x][:, :],
                                            in1=k8[:, :], op=ALU.is_ge)
                    nc.vector.copy_predicated(out=lo[x][:, :], mask=ge[x][:, :],
                                              data=tmid[x][:, :])
                    nc.vector.tensor_scalar(out=gei[x][:, :], in0=ge[x][:, :],
                                            scalar1=1, scalar2=None,
                                            op0=ALU.bitwise_xor)
                    nc.vector.copy_predicated(out=hi[x][:, :], mask=gei[x][:, :],
                                              data=tmid[x][:, :])

        # ---- exact pass at t* = lo ----
        if STAGE >= 6:
            for x, colk in (("T", COL_TKT), ("S", COL_TKS)):
                bcast_rows(lo[x], x)
                row_counts(x, lcm[x], cnt8[x])
                nc.vector.tensor_tensor(out=lse[x][:, :], in0=lcm[x][:, :],
                                        in1=sjunk[x][:, :], op=ALU.mult)
                nc.vector.tensor_reduce(
                    out=s8p[x][:, :],
                    in_=lse[x][:, :].rearrange("p (r t) -> p r t", r=R),
                    axis=mybir.AxisListType.X, op=ALU.add)
                psC = psum.tile([8, 2], F32, name="psC", tag="ps")
                nc.tensor.matmul(psC[:, 0:1], lhsT=cnt8[x][:, :],
                                 rhs=ones_sb[:, :], start=True, stop=True)
                nc.tensor.matmul(psC[:, 1:2], lhsT=s8p[x][:, :],
                                 rhs=ones_sb[:, :], start=True, stop=True)
                nc.vector.tensor_copy(out=ns8[x][:, :], in_=psC[:, :])
                # topk = S* + (k - n*) * t*
                nc.vector.tensor_tensor(out=tk[x][:, :], in0=k8[:, :],
                                        in1=ns8[x][:, 0:1], op=ALU.subtract)
                nc.vector.tensor_tensor(out=tk[x][:, :], in0=tk[x][:, :],
                                        in1=lo[x][:, :], op=ALU.mult)
                nc.vector.tensor_tensor(out=tk[x][:, :], in0=tk[x][:, :],
                                        in1=ns8[x][:, 1:2], op=ALU.add)
                nc.vector.tensor_scalar(out=tk[x][:, :], in0=tk[x][:, :],
                                        scalar1=0.5, scalar2=None, op0=ALU.mult)
                nc.vector.tensor_copy(out=partials[0:8, colk:colk + 1],
                                      in_=tk[x][:, :])

        # ---- conf_t flat layout (for loc masking) ----
        # gates: loc DMAs wait for end-of-streaming; the loc vector chain
        # waits for the search to finish so it fills the tail instead of
        # stalling the vector FIFO mid-kernel.
        if STAGE >= 6:
            for n in ("T", "S", "t"):
                nc.vector.tensor_copy(out=locsb[n][0:8, 0, 0:1],
                                      in_=lcm["S"][0:8, TCOL - 1:TCOL])
            nc.vector.tensor_copy(out=ld[0:8, 0:1], in_=tk["S"][:, :])
        nc.sync.dma_start(
            out=ctfl_i[:, :],
            in_=ctp.ap().rearrange("(p t) -> p t", t=LTT))
        nc.vector.tensor_copy(out=ctfl[:, :], in_=ctfl_i[:, :])
        nc.vector.tensor_scalar(out=posml[:, :], in0=ctfl[:, :], scalar1=0.5,
                                scalar2=None, op0=ALU.is_gt)

        # ---- loc DMAs ----
        for name, param in (("T", loc_T), ("S", loc_S), ("t", loc_t)):
            dst = locsb[name]
            flat = param.ap()
            for a in range(2):
                nc.sync.dma_start(
                    out=dst[:, a, 0:1092],
                    in_=flat[:, :]
                        .rearrange("(p a j) f -> p a (j f)", a=2, j=273)[:, a, :])

        # ---- loc smooth-L1 (masked, sum) ----
        # per element: 0.5*min(u,1)^2 + max(u,1) - 1 with u = |loc - loc_t|*pos
        # masked/pad elements contribute exactly +1, subtracted as NE_CONST.
        if STAGE >= 2:
            posml4 = (posml[:, :].rearrange("p (a j) -> p a j", a=2)
                      .unsqueeze(3).broadcast_to((128, 2, 273, 4)))
            for x, col in (("T", COL_LT), ("S", COL_LS)):
                nc.vector.tensor_tensor(
                    out=ld[:, :].rearrange("p (a e) -> p a e", a=2),
                    in0=locsb[x][:, :, 0:1092],
                    in1=locsb["t"][:, :, 0:1092],
                    op=ALU.subtract)
                nc.vector.tensor_tensor(
                    out=lu[:, :].rearrange("p (a j f) -> p a j f", a=2, j=273),
                    in0=ld[:, :].rearrange("p (a j f) -> p a j f", a=2, j=273),
                    in1=posml4, op=ALU.mult)
                nc.scalar.activation(out=lu[:, :], in_=lu[:, :], func=ACT.Abs)
                nc.vector.tensor_scalar(out=lc_[:, :], in0=lu[:, :], scalar1=1.0,
                                        scalar2=None, op0=ALU.min)
                nc.vector.tensor_scalar(out=lm[:, :], in0=lu[:, :], scalar1=1.0,
                                        scalar2=None, op0=ALU.max)
                nc.scalar.activation(out=lc_[:, :], in_=lc_[:, :], func=ACT.Square,
                                     scale=float(1.0 / np.sqrt(2.0)))
                nc.vector.tensor_tensor(out=ld[:, :], in0=lc_[:, :],
                                        in1=lm[:, :], op=ALU.add)
                nc.vector.tensor_reduce(out=partials[:, col:col + 1],
                                        in_=ld[:, :],
                                        axis=mybir.AxisListType.X, op=ALU.add)


        # ---- final partition reduce of partials -> out ----
        psF = psum.tile([1, NPART], F32, name="psF", tag="ps")
        nc.tensor.matmul(psF[:, :], lhsT=ones_sb[:, :], rhs=partials[:, :],
                         start=True, stop=True)
        fin = small.tile([1, NPART], F32)
        nc.vector.tensor_copy(out=fin[:, :], in_=psF[:, :])
        nc.sync.dma_start(out=out_p.ap(), in_=fin[:, :])
    nc.finalize()
    return nc


_NC_CACHE = None


def _get_nc():
    global _NC_CACHE
    if _NC_CACHE is None:
        _NC_CACHE = build_nc()
    return _NC_CACHE


def _host_consts():
    iota = np.ascontiguousarray(
        np.tile(np.arange(C, dtype=np.float32), FBT)[None, :].repeat(128, 0))
    ones8w = np.ones((8, 128), np.float32)
    eye81 = np.eye(81, dtype=np.float32)
    ones = np.ones((128, 1), np.float32)
    return iota, ones8w, eye81, ones


def _build_in_maps(inputs):
    conf_T = np.ascontiguousarray(np.asarray(inputs["conf_dataT"], np.float32))
    conf_S = np.ascontiguousarray(np.asarray(inputs["conf_dataS"], np.float32))
    loc_T = np.ascontiguousarray(np.asarray(inputs["loc_dataT"], np.float32))
    loc_S = np.ascontiguousarray(np.asarray(inputs["loc_dataS"], np.float32))
    loc_t = np.ascontiguousarray(np.asarray(inputs["loc_t"], np.float32))
    ct = np.ascontiguousarray(np.asarray(inputs["conf_t"], np.int32))
    PADN = 128 * LTT - R * P

    def _padloc(a):
        flat = a.reshape(R * P, 4)
        return np.ascontiguousarray(
            np.pad(flat, ((0, PADN), (0, 0))))
    iota, ones8w, eye81, ones = _host_consts()
    in_maps = []
    for d in range(NCORES):
        sl = slice(d * R, (d + 1) * R)
        ctsl = ct[sl]
        in_maps.append({
            "conf_T": conf_T[sl], "conf_S": conf_S[sl],
            "loc_T": _padloc(loc_T[sl]), "loc_S": _padloc(loc_S[sl]),
            "loc_t": _padloc(loc_t[sl]),
            "conf_t": ctsl,
            "ctp": np.ascontiguousarray(
                np.pad(ctsl.ravel(), (0, PADN), constant_values=-1)),
            "iota": iota, "ones8w": ones8w,
            "eye81": eye81, "ones128": ones,
        })
    return in_maps


def _combine(parts):
    S = parts.astype(np.float64).sum(axis=0)
    loss_cT = S[COL_AT] - S[COL_BT] + S[COL_CT] / 2 - S[COL_DT] + S[COL_TKT]
    loss_cS = S[COL_AS] - S[COL_BS] + S[COL_CS] / 2 - S[COL_DS] + S[COL_TKS]
    loss_lT = S[COL_LT] - NCORES * NE_CONST
    loss_lS = S[COL_LS] - NCORES * NE_CONST
    N = S[COL_NP]
    return np.array([loss_lT / N, loss_cT / N, loss_lS / N, loss_cS / N],
                    np.float32)


def run_on_hw(inputs, trace=False, **kw):
    nc = _get_nc()
    in_maps = _build_in_maps(inputs)
    res = run_bass_kernel_spmd(nc, in_maps, core_ids=list(range(NCORES)),
                               trace=trace, **kw)
    parts = np.stack([np.asarray(r["out"]).reshape(NPART) for r in res.results])
    return _combine(parts), res


def kernel(**inputs) -> np.ndarray:
    out, _ = run_on_hw(inputs, trace=False)
    return out
